# revision 1
# baseline (speedup 1.0000x reference)
"""AveragedNormals on 8 Trainium2 NeuronCores — single-sync design.

The axon tunnel costs ~105ms per host<->device synchronization regardless of
payload; chained dispatches and multi-array pulls amortize to one sync. So the
kernel does ONE pmap dispatch and ONE device_get, with the whole
KNN -> SHOT-LRF -> sign-vote -> neighbor-averaging pipeline on device, and the
host only fixing rows whose sign is decided by LAPACK's arbitrary eigenvector
sign convention.

Sharding: batch dim (2 samples) x 4-way query-row split = 8 shards; each core
holds its sample's full cloud and its 2048 query rows. Signed normals are
replicated within each sample's 4-core group via a masked psum, then each core
averages its rows' neighbor normals on device.

Correctness model (vs reference = top_k + LAPACK eigh + vote + gather-mean):
- The device normal z0 (closed-form 3x3 eigensolve + 2 inverse-iteration
  refinements) matches eigh's axis to ~1e-6 except near-degenerate eigengaps.
- The vote `pos >= neg` keeps the INPUT sign on ties, so rows with
  margin = 2*pos-K in [0, 2*zeta] (zeta = #exact-zero projections, >= 1 from
  self; margin even => usually {0,2}, ~10% of rows) resolve to LAPACK's
  arbitrary sign: the host runs numpy eigh on the pulled cov for exactly those
  rows and remaps the device vote counts (pos(-z) = neg(z) + zeta).
- Rows where the counts themselves are unstable (some |zp| < 1e-4*radius near
  a decision boundary, zeta > 1, or eigengap ratio < 1e-2) get their top-K
  index row exported so the host can recount the vote with the LAPACK vector.
- Each flagged row m contributes a correction delta_m = z_final - z_device to
  every row whose neighborhood contains m; the device exports per-row lists of
  flagged neighbors (cap TL=40, P(overflow) ~ 1e-8/row) so the host applies
  corrections to the pulled neighbor sums without the 4.2MB index pull.

Walrus constraints: indirect loads must stay <= 65536 indices per op (chunked
gathers with optimization_barrier so XLA can't re-fuse them); mhlo.acos does
not lower (atan2 form instead).
"""

import functools
import os
import time

import jax
import jax.numpy as jnp
import numpy as np
from jax import lax

_DEBUG_T = bool(os.environ.get("AN_DEBUG_T"))
_tmarks = []
_row0_dev = None

B = 2
N = 8192
K = 128
SPLIT = 4  # row-split per sample
NC = 8
ROWS = N // SPLIT  # 2048
EPS = 1e-12
TL = 44  # per-row flagged-neighbor list capacity (measured max 37 on the fixed input)
HARD = 48  # per-core exported hard-row (recount) capacity (measured max 29)
HI = lax.Precision.HIGHEST
GROUPS = [[0, 1, 2, 3], [4, 5, 6, 7]]


def _dist(vq, v_full):
    sq_all = jnp.sum(v_full * v_full, axis=-1)
    sq_q = jnp.sum(vq * vq, axis=-1)
    dot = lax.dot_general(vq, v_full, (((1,), (1,)), ((), ())), precision=HI)
    d2 = sq_q[:, None] - 2.0 * dot + sq_all[None, :]
    return jnp.sqrt(jnp.maximum(d2, EPS))  # [ROWS, N]


def _chunked_gather(table, idx, nchunks):
    # Walrus overflows a 16-bit semaphore field on >~65K-index IndirectLoads,
    # and XLA re-fuses naive chunked gathers of contiguous index slices back
    # into one op. The optimization_barrier on each index chunk hides the
    # contiguity, keeping the gathers separate (<=65536 indices each).
    parts = []
    step = idx.shape[0] // nchunks
    for c in range(nchunks):
        ix = lax.optimization_barrier(idx[c * step : (c + 1) * step])
        parts.append(table[ix])
    return jnp.concatenate(parts, axis=0)


def _smallest_evec_gap(cov):
    # cov: [R, 3, 3] symmetric. Unit eigenvector of the smallest eigenvalue
    # plus the relative gap (lam_mid - lam_min) / (lam_max - lam_min).
    a00 = cov[:, 0, 0]
    a01 = cov[:, 0, 1]
    a02 = cov[:, 0, 2]
    a11 = cov[:, 1, 1]
    a12 = cov[:, 1, 2]
    a22 = cov[:, 2, 2]

    q = (a00 + a11 + a22) / 3.0
    b00 = a00 - q
    b11 = a11 - q
    b22 = a22 - q
    p1 = a01 * a01 + a02 * a02 + a12 * a12
    p2 = b00 * b00 + b11 * b11 + b22 * b22 + 2.0 * p1
    p = jnp.sqrt(jnp.maximum(p2 / 6.0, 1e-30))
    detb = (
        b00 * (b11 * b22 - a12 * a12)
        - a01 * (a01 * b22 - a12 * a02)
        + a02 * (a01 * a12 - b11 * a02)
    )
    r = jnp.clip(detb / (2.0 * p * p * p), -1.0, 1.0)
    # acos via atan2 (mhlo.acos doesn't lower on the neuron backend)
    phi = jnp.arctan2(jnp.sqrt(jnp.maximum(1.0 - r * r, 0.0)), r) / 3.0
    lam_hi = q + 2.0 * p * jnp.cos(phi)
    lam = q + 2.0 * p * jnp.cos(phi + 2.0 * np.pi / 3.0)  # smallest
    lam_mid = 3.0 * q - lam_hi - lam
    spread = jnp.maximum(lam_hi - lam, 1e-30)
    gapr = (lam_mid - lam) / spread

    m00 = a00 - lam
    m11 = a11 - lam
    m22 = a22 - lam
    r0 = jnp.stack([m00, a01, a02], axis=-1)
    r1 = jnp.stack([a01, m11, a12], axis=-1)
    r2 = jnp.stack([a02, a12, m22], axis=-1)
    c01 = jnp.cross(r0, r1)
    c02 = jnp.cross(r0, r2)
    c12 = jnp.cross(r1, r2)
    n01 = jnp.sum(c01 * c01, axis=-1)
    n02 = jnp.sum(c02 * c02, axis=-1)
    n12 = jnp.sum(c12 * c12, axis=-1)
    best12 = (n12 >= n01) & (n12 >= n02)
    best02 = (n02 >= n01) & ~best12
    v = jnp.where(best12[:, None], c12, jnp.where(best02[:, None], c02, c01))
    nv = jnp.sqrt(jnp.maximum(jnp.sum(v * v, axis=-1, keepdims=True), 1e-30))
    v = v / nv

    # Two inverse-iteration refinements (Rayleigh quotient + adjugate solve):
    # the closed-form z is only ~1e-3 accurate; the vote is decided by
    # near-zero neighbor projections, so z must match eigh to ~1e-6.
    eps_reg = 1e-7 * jnp.maximum(jnp.abs(q), p)
    for _ in range(2):
        lam_r = (
            v[:, 0] * (a00 * v[:, 0] + a01 * v[:, 1] + a02 * v[:, 2])
            + v[:, 1] * (a01 * v[:, 0] + a11 * v[:, 1] + a12 * v[:, 2])
            + v[:, 2] * (a02 * v[:, 0] + a12 * v[:, 1] + a22 * v[:, 2])
        )
        m00 = a00 - lam_r + eps_reg
        m11 = a11 - lam_r + eps_reg
        m22 = a22 - lam_r + eps_reg
        y0 = (
            (m11 * m22 - a12 * a12) * v[:, 0]
            + (a02 * a12 - a01 * m22) * v[:, 1]
            + (a01 * a12 - a02 * m11) * v[:, 2]
        )
        y1 = (
            (a02 * a12 - a01 * m22) * v[:, 0]
            + (m00 * m22 - a02 * a02) * v[:, 1]
            + (a01 * a02 - m00 * a12) * v[:, 2]
        )
        y2 = (
            (a01 * a12 - a02 * m11) * v[:, 0]
            + (a01 * a02 - m00 * a12) * v[:, 1]
            + (m00 * m11 - a01 * a01) * v[:, 2]
        )
        y = jnp.stack([y0, y1, y2], axis=-1)
        y = jnp.where(jnp.sum(y * v, axis=-1, keepdims=True) < 0, -y, y)
        ny = jnp.sqrt(jnp.maximum(jnp.sum(y * y, axis=-1, keepdims=True), 1e-38))
        v = y / ny
    return v, gapr


@functools.partial(jax.pmap, axis_name="i")
def _stage1(v_sh, row0):
    # v_sh: [ROWS, 3] this core's query block; replicate the full cloud via a
    # group psum of disjoint zero-padded blocks (bit-identical, 4x less push)
    vq = v_sh
    vfp = jnp.zeros((N, 3), jnp.float32)
    vfp = lax.dynamic_update_slice(vfp, v_sh, (row0[0], 0))
    v_full = lax.psum(vfp, "i", axis_index_groups=GROUPS)
    d = _dist(vq, v_full)  # [ROWS, N]
    neg_d, idx = lax.top_k(-d, K)
    radius = -neg_d[:, -1]  # [ROWS] distance to 128th-nearest (incl. self)

    # direct gathered neighborhoods: same arithmetic path as the reference
    nbh = _chunked_gather(v_full, idx, 4) - vq[:, None, :]  # [ROWS, K, 3]
    dn = jnp.sqrt(jnp.maximum(jnp.sum(nbh * nbh, axis=-1), EPS))  # [ROWS, K]
    w = radius[:, None] - dn
    wn = w[:, :, None] * nbh
    cov = lax.dot_general(
        jnp.swapaxes(wn, 1, 2), nbh, (((2,), (1,)), ((0,), (0,))), precision=HI
    )  # [ROWS, 3, 3]
    cov = cov / jnp.sum(w, axis=-1)[:, None, None]

    z0, gapr = _smallest_evec_gap(cov)  # [ROWS, 3], [ROWS]

    # SHOT sign vote with the device eigenvector
    zp = jnp.sum(nbh * z0[:, None, :], axis=-1)  # [ROWS, K]
    posc = jnp.sum((zp >= 0).astype(jnp.int32), axis=-1)
    zeta = jnp.sum((zp == 0).astype(jnp.int32), axis=-1)
    margin = 2 * posc - K
    s = jnp.where(margin >= 0, 1.0, -1.0).astype(jnp.float32)
    zs = s[:, None] * z0  # vote-oriented device normal

    # ambiguity flags (host fixes these rows with LAPACK eigh)
    abszp = jnp.where(zp == 0, jnp.float32(np.inf), jnp.abs(zp))
    minabs = jnp.min(abszp, axis=-1)
    f_tie = (margin >= 0) & (margin <= 2 * zeta)  # LAPACK sign decides
    f_zp = (
        (minabs < 3e-5 * radius) & (margin >= -4) & (margin <= 2 * zeta + 4)
    ) | (zeta > 1)  # counts unstable near a boundary (z0 error ~1e-6)
    f_gap = gapr < 3e-3  # device eigenvector unreliable
    recount = f_zp | f_gap
    flag = f_tie.astype(jnp.int32) + 2 * recount.astype(jnp.int32)

    # replicate signed normals + flags across the sample's 4-core group
    # (packed into one [N,4] collective: collectives are latency-bound here)
    zf = jnp.concatenate([zs, (flag > 0).astype(jnp.float32)[:, None]], axis=1)
    zfull = jnp.zeros((N, 4), jnp.float32)
    zfull = lax.dynamic_update_slice(zfull, zf, (row0[0], 0))
    zfull = lax.psum(zfull, "i", axis_index_groups=GROUPS)

    # one combined neighbor gather: normals sum + flagged-neighbor mask
    g = _chunked_gather(zfull, idx, 4)  # [ROWS, K, 4]
    S = jnp.sum(g[:, :, :3], axis=1)  # [ROWS, 3]
    fl = g[:, :, 3]  # [ROWS, K]
    nflg = jnp.sum((fl > 0).astype(jnp.int32), axis=-1)  # flagged-nbr count
    # f32 scores: neuron TopK rejects integer inputs; values < 2^24 are exact
    score = jnp.where(fl > 0, (idx + N).astype(jnp.float32), 0.0)
    tlv, _ = lax.top_k(score, TL)
    tlv = tlv.astype(jnp.int32)
    tielist = jnp.where(tlv >= N, tlv - N, -1).astype(jnp.int16)

    # export top-K index rows for rows needing a host vote recount
    hsc = recount.astype(jnp.float32) * 100000.0 + jnp.arange(
        ROWS, dtype=jnp.float32
    )
    hval, hrow = lax.top_k(hsc, HARD)
    hard_rows = jnp.where(hval >= 100000.0, hrow, -1).astype(jnp.int16)
    hard_idx = jnp.take(idx, hrow, axis=0).astype(jnp.int16)  # [HARD, K]

    # int8 aux: margin is even (store margin/2 in [-64,64]); clip counts to 127
    aux = jnp.stack(
        [
            margin // 2,
            jnp.minimum(zeta, 127),
            flag,
            jnp.minimum(nflg, 127),
        ],
        axis=-1,
    ).astype(jnp.int8)
    # 6 components; LOWER triangle entries: np.linalg.eigh reads the lower
    # triangle, and cov[1,0] vs cov[0,1] can differ in the last bit, which
    # flips LAPACK's arbitrary sign on tie rows. Match the baseline exactly.
    cov6 = jnp.stack(
        [
            cov[:, 0, 0],
            cov[:, 1, 1],
            cov[:, 2, 2],
            cov[:, 1, 0],
            cov[:, 2, 0],
            cov[:, 2, 1],
        ],
        axis=-1,
    )
    return cov6, zs, S, aux, tielist, hard_rows, hard_idx


def kernel(vertices: np.ndarray) -> np.ndarray:
    vertices = np.asarray(vertices, dtype=np.float32)
    assert vertices.shape == (B, N, 3)
    v_sh = vertices.reshape(NC, ROWS, 3)  # core c -> sample c//4, block c%4
    row0 = np.array([[(c % SPLIT) * ROWS] for c in range(NC)], dtype=np.int32)

    t0 = time.perf_counter()
    global _row0_dev
    if _row0_dev is None:
        _row0_dev = jnp.asarray(row0)
    outs = _stage1(jnp.asarray(v_sh), _row0_dev)
    t1 = time.perf_counter()
    cov6, zs, S, aux, tielist, hard_rows, hard_idx = jax.device_get(outs)  # one sync
    t2 = time.perf_counter()
    global _last_debug
    _last_debug = (aux, tielist, hard_rows)

    _tmarks.clear()
    tp = time.perf_counter()

    def _mark(name):
        nonlocal tp
        now = time.perf_counter()
        _tmarks.append((name, now - tp))
        tp = now

    # core c -> sample c//4, rows [(c%4)*ROWS, ...): plain reshape restores [B,N]
    c6 = cov6.reshape(B, N, 6)
    covg = np.empty((B, N, 3, 3), np.float32)
    covg[..., 0, 0] = c6[..., 0]
    covg[..., 1, 1] = c6[..., 1]
    covg[..., 2, 2] = c6[..., 2]
    covg[..., 0, 1] = covg[..., 1, 0] = c6[..., 3]  # device cov[1,0]
    covg[..., 0, 2] = covg[..., 2, 0] = c6[..., 4]  # device cov[2,0]
    covg[..., 1, 2] = covg[..., 2, 1] = c6[..., 5]  # device cov[2,1]
    zsg = zs.reshape(B, N, 3)
    Sg = np.array(S.reshape(B, N, 3))  # writable copy (device_get is read-only)
    auxg = aux.reshape(B, N, 4).astype(np.int32)
    margin = 2 * auxg[..., 0]
    zeta = auxg[..., 1]
    flag = auxg[..., 2]
    tl = tielist.reshape(B, N, TL)
    _mark("unpack")

    # hard-row exports (vectorized): per-core flagged slots sort first
    nhard = (hard_rows >= 0).sum(axis=1)  # [NC]
    hard_maps = []  # per sample: row -> slot in hidx_all
    hidx_all = []
    for b in range(B):
        rows_g, idxs = [], []
        for c in range(b * SPLIT, (b + 1) * SPLIT):
            n = int(nhard[c])
            rows_g.append(hard_rows[c, :n].astype(np.int32) + (c % SPLIT) * ROWS)
            idxs.append(hard_idx[c, :n])
        rows_g = np.concatenate(rows_g)
        lut = np.full(N, -1, np.int32)
        lut[rows_g] = np.arange(rows_g.size, dtype=np.int32)
        hard_maps.append(lut)
        hidx_all.append(np.concatenate(idxs).astype(np.int32))
    _mark("hardmap")

    for b in range(B):
        rows = np.nonzero(flag[b])[0]
        if rows.size == 0:
            continue
        # LAPACK eigh only on ambiguous rows: its sign convention is the spec
        _, vecs = np.linalg.eigh(covg[b][rows])
        zl = np.ascontiguousarray(vecs[:, :, 0])  # [R, 3]
        _mark(f"eigh{b}")
        mg = margin[b][rows]
        z0 = np.where(mg >= 0, 1.0, -1.0).astype(np.float32)[:, None] * zsg[b][rows]
        # remap device counts to the LAPACK orientation: pos(-z) = neg(z) + zeta
        sigma = np.einsum("rc,rc->r", zl, z0)
        pos = np.where(sigma >= 0, (mg + K) // 2, (K - mg) // 2 + zeta[b][rows])
        # rows needing a true recount (unstable counts / unreliable device vec)
        rc = np.nonzero((flag[b][rows] >= 2) & (hard_maps[b][rows] >= 0))[0]
        if rc.size:
            slots = hard_maps[b][rows[rc]]
            nb = vertices[b][hidx_all[b][slots]] - vertices[b][rows[rc], None, :]
            zp = np.einsum("rkc,rc->rk", nb, zl[rc])
            pos[rc] = (zp >= 0).sum(axis=1)
        final = np.where((2 * pos - K >= 0)[:, None], zl, -zl)
        delta = final - zsg[b][rows]
        _mark(f"vote{b}")
        # apply corrections to every row whose neighborhood has a flagged row.
        # top_k sorts valid (score>0) entries first, so each row's valid
        # entries are a prefix of length nflg: no boolean scan needed.
        nf = np.minimum(auxg[b, :, 3], TL)  # clip: over-cap degrades, not crashes
        rows_i = np.repeat(np.arange(N, dtype=np.int32), nf)
        tlb = tl[b]
        cols = tlb[np.arange(TL)[None, :] < nf[:, None]].astype(np.int32)
        dlut = np.zeros((N, 3), np.float32)
        dlut[rows] = delta
        dv = dlut[cols]
        for c in range(3):
            Sg[b, :, c] += np.bincount(rows_i, weights=dv[:, c], minlength=N)
        _mark(f"corr{b}")

    out = Sg / np.linalg.norm(Sg, axis=-1, keepdims=True)
    if _DEBUG_T:
        t3 = time.perf_counter()
        print(
            f"[kernel] dispatch {(t1-t0)*1e3:.1f}ms  sync+pull {(t2-t1)*1e3:.1f}ms"
            f"  host-fix {(t3-t2)*1e3:.1f}ms  "
            + " ".join(f"{k}={v*1e3:.1f}" for k, v in _tmarks),
            flush=True,
        )
    return out.astype(np.float32)



# revision 2
# speedup vs baseline: 1.1240x; 1.1240x over previous
"""AveragedNormals on 8 Trainium2 NeuronCores — gather-free single-sync design.

Tunnel model (measured): every host<->device sync costs a fixed ~40-85ms RTT
(network-dependent), pull bandwidth ~77MB/s, chained dispatches are free.
Device-side indirect gathers are the other big cost (~55ms for the baseline's
two [2048x128] gathers), while lax.top_k on [2048, 8192] is only ~6ms.

So this kernel removes ALL device gathers by exploiting the SHOT weight
structure: w_j = radius - d_j is >= 0 exactly for the 128 nearest neighbors
and the weight of the 128th is exactly 0, so

  cov_n  = sum_j relu(radius_n - d_nj) x_nj x_nj^T   (x = v_j - q_n)
  vote_n = sum_j [d2_nj <= r2k_n] f(x_nj . z_n)
  S_n    = sum_j [d2_nj <= r2k_n] zsigned_j          (masked matmul)

over ALL 8192 points — identical term sets to the reference's gathered top-128
versions (only fp summation order differs, ~1e-7). Only the 128th-smallest
distance r2k per row is needed (top_k values; the index matrix is used solely
for the small hard-row export).

Correctness model (vs reference = top_k + LAPACK eigh + vote + gather-mean),
same as the baseline: ambiguous rows (vote ties in [0, 2*zeta], unstable
counts, weak eigengap, mask-count != 128) are exported compactly (cov6 + zs +
aux for <=448 rows/core) and fixed on host with np.linalg.eigh; corrections
delta_m = z_final - z_device propagate to every row whose neighborhood holds m
via a HOST-side distance matmul (vertices @ flipped^T vs pulled r2k) instead of
pulling per-row neighbor lists — cutting the pull from ~2.4MB to ~0.5MB.
"""

import functools
import os
import time

import jax
import jax.numpy as jnp
import numpy as np
from jax import lax

_DEBUG_T = bool(os.environ.get("AN_DEBUG_T"))
_DEBUG_FULL = bool(os.environ.get("AN_DEBUG_FULL"))
_tmarks = []

B = 2
N = 8192
K = 128
SPLIT = 4  # row-split per sample
NC = 8
ROWS = N // SPLIT  # 2048
EPS = 1e-12
FCAP = 256  # per-core flagged-row export capacity (observed max 155)
HARD = 48  # per-core exported hard-row (recount) capacity (measured max ~29)
HI = lax.Precision.HIGHEST
GROUPS = [[0, 1, 2, 3], [4, 5, 6, 7]]


def _smallest_evec_gap(cov):
    # cov: [R, 3, 3] symmetric. Unit eigenvector of the smallest eigenvalue
    # plus the relative gap (lam_mid - lam_min) / (lam_max - lam_min).
    a00 = cov[:, 0, 0]
    a01 = cov[:, 0, 1]
    a02 = cov[:, 0, 2]
    a11 = cov[:, 1, 1]
    a12 = cov[:, 1, 2]
    a22 = cov[:, 2, 2]

    q = (a00 + a11 + a22) / 3.0
    b00 = a00 - q
    b11 = a11 - q
    b22 = a22 - q
    p1 = a01 * a01 + a02 * a02 + a12 * a12
    p2 = b00 * b00 + b11 * b11 + b22 * b22 + 2.0 * p1
    p = jnp.sqrt(jnp.maximum(p2 / 6.0, 1e-30))
    detb = (
        b00 * (b11 * b22 - a12 * a12)
        - a01 * (a01 * b22 - a12 * a02)
        + a02 * (a01 * a12 - b11 * a02)
    )
    r = jnp.clip(detb / (2.0 * p * p * p), -1.0, 1.0)
    # acos via atan2 (mhlo.acos doesn't lower on the neuron backend)
    phi = jnp.arctan2(jnp.sqrt(jnp.maximum(1.0 - r * r, 0.0)), r) / 3.0
    lam_hi = q + 2.0 * p * jnp.cos(phi)
    lam = q + 2.0 * p * jnp.cos(phi + 2.0 * np.pi / 3.0)  # smallest
    lam_mid = 3.0 * q - lam_hi - lam
    spread = jnp.maximum(lam_hi - lam, 1e-30)
    gapr = (lam_mid - lam) / spread

    m00 = a00 - lam
    m11 = a11 - lam
    m22 = a22 - lam
    r0 = jnp.stack([m00, a01, a02], axis=-1)
    r1 = jnp.stack([a01, m11, a12], axis=-1)
    r2 = jnp.stack([a02, a12, m22], axis=-1)
    c01 = jnp.cross(r0, r1)
    c02 = jnp.cross(r0, r2)
    c12 = jnp.cross(r1, r2)
    n01 = jnp.sum(c01 * c01, axis=-1)
    n02 = jnp.sum(c02 * c02, axis=-1)
    n12 = jnp.sum(c12 * c12, axis=-1)
    best12 = (n12 >= n01) & (n12 >= n02)
    best02 = (n02 >= n01) & ~best12
    v = jnp.where(best12[:, None], c12, jnp.where(best02[:, None], c02, c01))
    nv = jnp.sqrt(jnp.maximum(jnp.sum(v * v, axis=-1, keepdims=True), 1e-30))
    v = v / nv

    # Two inverse-iteration refinements (Rayleigh quotient + adjugate solve):
    # the closed-form z is only ~1e-3 accurate; the vote is decided by
    # near-zero neighbor projections, so z must match eigh to ~1e-6.
    eps_reg = 1e-7 * jnp.maximum(jnp.abs(q), p)
    for _ in range(2):
        lam_r = (
            v[:, 0] * (a00 * v[:, 0] + a01 * v[:, 1] + a02 * v[:, 2])
            + v[:, 1] * (a01 * v[:, 0] + a11 * v[:, 1] + a12 * v[:, 2])
            + v[:, 2] * (a02 * v[:, 0] + a12 * v[:, 1] + a22 * v[:, 2])
        )
        m00 = a00 - lam_r + eps_reg
        m11 = a11 - lam_r + eps_reg
        m22 = a22 - lam_r + eps_reg
        y0 = (
            (m11 * m22 - a12 * a12) * v[:, 0]
            + (a02 * a12 - a01 * m22) * v[:, 1]
            + (a01 * a12 - a02 * m11) * v[:, 2]
        )
        y1 = (
            (a02 * a12 - a01 * m22) * v[:, 0]
            + (m00 * m22 - a02 * a02) * v[:, 1]
            + (a01 * a02 - m00 * a12) * v[:, 2]
        )
        y2 = (
            (a01 * a12 - a02 * m11) * v[:, 0]
            + (a01 * a02 - m00 * a12) * v[:, 1]
            + (m00 * m11 - a01 * a01) * v[:, 2]
        )
        y = jnp.stack([y0, y1, y2], axis=-1)
        y = jnp.where(jnp.sum(y * v, axis=-1, keepdims=True) < 0, -y, y)
        ny = jnp.sqrt(jnp.maximum(jnp.sum(y * y, axis=-1, keepdims=True), 1e-38))
        v = y / ny
    return v, gapr


@functools.partial(jax.pmap, axis_name="i")
def _stage1(v_sh, row0):
    # v_sh: [ROWS, 3] this core's query block; replicate the full cloud via a
    # group psum of disjoint zero-padded blocks (bit-identical, 4x less push)
    vq = v_sh
    vfp = jnp.zeros((N, 3), jnp.float32)
    vfp = lax.dynamic_update_slice(vfp, v_sh, (row0[0], 0))
    v_full = lax.psum(vfp, "i", axis_index_groups=GROUPS)

    # d2 via ELEMENTWISE ops in a fixed association order, bitwise-reproducible
    # by numpy on the host: the host correction step must replicate the
    # device's neighborhood-membership test d2 <= r2k exactly, and a matmul's
    # accumulation order can't be reproduced host-side. Mirror this exact
    # op sequence (products, left-assoc sums) in kernel()'s containment test.
    a0 = v_full[:, 0]
    a1 = v_full[:, 1]
    a2 = v_full[:, 2]
    sq_all = (a0 * a0 + a1 * a1) + a2 * a2  # [N]
    q0 = vq[:, 0]
    q1 = vq[:, 1]
    q2 = vq[:, 2]
    sq_q = (q0 * q0 + q1 * q1) + q2 * q2  # [ROWS]
    dot = (q0[:, None] * a0[None, :] + q1[:, None] * a1[None, :]) + q2[:, None] * a2[None, :]
    d2 = (sq_q[:, None] - 2.0 * dot) + sq_all[None, :]  # [ROWS, N]
    # One materialization of d2 shared by top_k / mask / weights: XLA would
    # otherwise rematerialize it per consumer fusion with different rounding,
    # silently swapping boundary neighbors between the top-k set and the mask.
    d2 = lax.optimization_barrier(d2)

    # sqrt is monotone, so top-k of -d2 selects the same neighbor set as the
    # reference's top-k of -sqrt(max(d2, EPS)) (the EPS plateau only merges
    # the self-distance, which is in the set either way).
    neg2, idx = lax.top_k(-d2, K)
    r2k = -neg2[:, -1]  # [ROWS] squared distance of the 128th-nearest
    radius = jnp.sqrt(jnp.maximum(r2k, EPS))

    # gather-free masked reductions over all N points
    x0 = v_full[:, 0][None, :] - vq[:, 0][:, None]  # [ROWS, N]
    x1 = v_full[:, 1][None, :] - vq[:, 1][:, None]
    x2 = v_full[:, 2][None, :] - vq[:, 2][:, None]
    d = jnp.sqrt(jnp.maximum(d2, EPS))
    w = jnp.maximum(radius[:, None] - d, 0.0)  # SHOT weight; 0 outside top-K
    mask = (d2 <= r2k[:, None]).astype(jnp.float32)  # [ROWS, N]

    cnt = jnp.sum(mask, axis=-1)  # 128 unless d2 ties at the boundary
    wsum = jnp.sum(w, axis=-1)
    c00 = jnp.sum(w * x0 * x0, axis=-1)
    c11 = jnp.sum(w * x1 * x1, axis=-1)
    c22 = jnp.sum(w * x2 * x2, axis=-1)
    c01 = jnp.sum(w * x0 * x1, axis=-1)
    c02 = jnp.sum(w * x0 * x2, axis=-1)
    c12 = jnp.sum(w * x1 * x2, axis=-1)
    iw = 1.0 / wsum
    cov6 = jnp.stack([c00, c11, c22, c01, c02, c12], axis=-1) * iw[:, None]
    cov = jnp.stack(
        [
            jnp.stack([c00, c01, c02], axis=-1),
            jnp.stack([c01, c11, c12], axis=-1),
            jnp.stack([c02, c12, c22], axis=-1),
        ],
        axis=1,
    ) * iw[:, None, None]

    z0, gapr = _smallest_evec_gap(cov)  # [ROWS, 3], [ROWS]

    # SHOT sign vote with the device eigenvector, masked over all N
    zp = x0 * z0[:, 0][:, None] + x1 * z0[:, 1][:, None] + x2 * z0[:, 2][:, None]
    posc = jnp.sum(mask * (zp >= 0), axis=-1).astype(jnp.int32)
    zeta = jnp.sum(mask * (zp == 0), axis=-1).astype(jnp.int32)
    abszp = jnp.where((mask > 0) & (zp != 0), jnp.abs(zp), jnp.float32(np.inf))
    minabs = jnp.min(abszp, axis=-1)
    margin = 2 * posc - K
    s = jnp.where(margin >= 0, 1.0, -1.0).astype(jnp.float32)
    zs = s[:, None] * z0  # vote-oriented device normal

    # ambiguity flags (host fixes these rows with LAPACK eigh)
    f_tie = (margin >= 0) & (margin <= 2 * zeta)  # LAPACK sign decides
    f_zp = (
        (minabs < 3e-5 * radius) & (margin >= -4) & (margin <= 2 * zeta + 4)
    ) | (zeta > 1)  # counts unstable near a boundary (z0 error ~1e-6)
    f_gap = gapr < 3e-3  # device eigenvector unreliable
    f_cnt = cnt != 128.0  # boundary tie: mask picked up extras
    recount = f_zp | f_gap | f_cnt
    flag = f_tie.astype(jnp.int32) + 2 * recount.astype(jnp.int32)

    # replicate signed normals across the sample's 4-core group
    zfull = jnp.zeros((N, 3), jnp.float32)
    zfull = lax.dynamic_update_slice(zfull, zs, (row0[0], 0))
    zfull = lax.psum(zfull, "i", axis_index_groups=GROUPS)

    # neighbor-average numerator via masked matmul (no gather)
    S = lax.dot_general(mask, zfull, (((1,), (0,)), ((), ())), precision=HI)

    # compacted flagged-row export (f32 scores: TopK rejects integer inputs)
    fsc = (flag > 0).astype(jnp.float32) * 100000.0 + jnp.arange(
        ROWS, dtype=jnp.float32
    )
    fval, frow = lax.top_k(fsc, FCAP)
    frows = jnp.where(fval >= 100000.0, frow, -1).astype(jnp.int16)
    fcov6 = jnp.take(cov6, frow, axis=0)  # [FCAP, 6]
    fzs = jnp.take(zs, frow, axis=0)  # [FCAP, 3]
    faux = jnp.stack(
        [
            jnp.take(margin, frow) // 2,  # margin even: store margin/2
            jnp.minimum(jnp.take(zeta, frow), 127),
            jnp.take(flag, frow),
            jnp.clip(jnp.take(cnt.astype(jnp.int32), frow) - K, -8, 8),
        ],
        axis=-1,
    ).astype(jnp.int8)

    # export top-K index rows for rows needing a host vote recount
    hsc = recount.astype(jnp.float32) * 100000.0 + jnp.arange(
        ROWS, dtype=jnp.float32
    )
    hval, hrow = lax.top_k(hsc, HARD)
    hard_rows = jnp.where(hval >= 100000.0, hrow, -1).astype(jnp.int16)
    hard_idx = jnp.take(idx, hrow, axis=0).astype(jnp.int16)  # [HARD, K]
    # exact top-K-set sum for hard rows (tiny gather): boundary-tie rows'
    # masked S picked up extras; the host swaps in this exact sum instead
    hard_S = jnp.take(zfull, jnp.take(idx, hrow, axis=0).reshape(-1), axis=0)
    hard_S = jnp.sum(hard_S.reshape(HARD, K, 3), axis=1)  # [HARD, 3]

    if _DEBUG_FULL:
        return S, r2k, frows, fcov6, fzs, faux, hard_rows, hard_idx, hard_S, zs, jnp.stack([margin, zeta, flag], -1), cov6, gapr, minabs, idx.astype(jnp.int16), cnt
    return S, r2k, frows, fcov6, fzs, faux, hard_rows, hard_idx, hard_S


_row0_dev = None
_corr_buf = None  # preallocated [N, FMAX] f32 scratch for the host correction
_corr_f32 = None
FMAX = 192  # max big-delta columns per sample (observed ~120)


def kernel(vertices: np.ndarray) -> np.ndarray:
    vertices = np.asarray(vertices, dtype=np.float32)
    assert vertices.shape == (B, N, 3)
    v_sh = vertices.reshape(NC, ROWS, 3)  # core c -> sample c//4, block c%4
    row0 = np.array([[(c % SPLIT) * ROWS] for c in range(NC)], dtype=np.int32)

    t0 = time.perf_counter()
    global _row0_dev
    if _row0_dev is None:
        _row0_dev = jnp.asarray(row0)
    outs = _stage1(jnp.asarray(v_sh), _row0_dev)
    t1 = time.perf_counter()
    pulled = jax.device_get(outs)
    S, r2k, frows, fcov6, fzs, faux, hard_rows, hard_idx, hard_S = pulled[:9]
    t2 = time.perf_counter()
    global _last_pull
    _last_pull = pulled

    _tmarks.clear()
    tp = time.perf_counter()

    def _mark(name):
        nonlocal tp
        now = time.perf_counter()
        _tmarks.append((name, now - tp))
        tp = now

    # core c -> sample c//4, rows [(c%4)*ROWS, ...): plain reshape restores [B,N]
    Sg = np.array(S.reshape(B, N, 3))  # writable copy
    r2g = r2k.reshape(B, N)
    # n2 mirrors the device's sq_all association order exactly
    n2 = (vertices[..., 0] * vertices[..., 0] + vertices[..., 1] * vertices[..., 1]) + vertices[..., 2] * vertices[..., 2]

    # hard-row maps (vectorized): per-core flagged slots sort first
    nhard = (hard_rows >= 0).sum(axis=1)  # [NC]
    _mark("unpack")

    for b in range(B):
        # gather this sample's flagged-row export across its 4 cores
        rows_l, cov_l, zs_l, aux_l = [], [], [], []
        hmap = np.full(N, -1, np.int32)
        hidx_l, hS_l = [], []
        hn = 0
        for c in range(b * SPLIT, (b + 1) * SPLIT):
            nv = int((frows[c] >= 0).sum())
            off = (c % SPLIT) * ROWS
            rows_l.append(frows[c, :nv].astype(np.int32) + off)
            cov_l.append(fcov6[c, :nv])
            zs_l.append(fzs[c, :nv])
            aux_l.append(faux[c, :nv])
            nh = int(nhard[c])
            hmap[hard_rows[c, :nh].astype(np.int32) + off] = hn + np.arange(nh)
            hidx_l.append(hard_idx[c, :nh].astype(np.int32))
            hS_l.append(hard_S[c, :nh])
            hn += nh
        rows = np.concatenate(rows_l)
        if _DEBUG_T:
            nv_pc = [int((frows[c] >= 0).sum()) for c in range(b * SPLIT, (b + 1) * SPLIT)]
            print(f"[kernel] sample {b}: flagged/core {nv_pc} (cap {FCAP}), hard {hn} (cap {HARD}/core)", flush=True)
        if rows.size == 0:
            continue
        cov6b = np.concatenate(cov_l).astype(np.float32)
        zsb = np.concatenate(zs_l).astype(np.float32)
        auxb = np.concatenate(aux_l).astype(np.int32)
        hidx_b = np.concatenate(hidx_l) if hn else np.zeros((0, K), np.int32)
        hS_b = np.concatenate(hS_l) if hn else np.zeros((0, 3), np.float32)
        mg = 2 * auxb[:, 0]
        zeta = auxb[:, 1]
        flag = auxb[:, 2]
        cntd = auxb[:, 3]  # cnt - 128 (boundary-tie rows have cntd != 0)
        _mark(f"gather{b}")

        covg = np.empty((rows.size, 3, 3), np.float32)
        covg[:, 0, 0] = cov6b[:, 0]
        covg[:, 1, 1] = cov6b[:, 1]
        covg[:, 2, 2] = cov6b[:, 2]
        covg[:, 0, 1] = covg[:, 1, 0] = cov6b[:, 3]
        covg[:, 0, 2] = covg[:, 2, 0] = cov6b[:, 4]
        covg[:, 1, 2] = covg[:, 2, 1] = cov6b[:, 5]
        # LAPACK eigh only on ambiguous rows: its sign convention is the spec
        _, vecs = np.linalg.eigh(covg)
        zl = np.ascontiguousarray(vecs[:, :, 0])  # [R, 3]
        _mark(f"eigh{b}")

        z0 = np.where(mg >= 0, 1.0, -1.0).astype(np.float32)[:, None] * zsb
        # remap device counts to the LAPACK orientation: pos(-z) = neg(z) + zeta
        sigma = np.einsum("rc,rc->r", zl, z0)
        pos = np.where(sigma >= 0, (mg + K) // 2, (K - mg) // 2 + zeta)
        # rows needing a true recount (unstable counts / unreliable device vec)
        rc = np.nonzero((flag >= 2) & (hmap[rows] >= 0))[0]
        if rc.size:
            slots = hmap[rows[rc]]
            nb = vertices[b][hidx_b[slots]] - vertices[b][rows[rc], None, :]
            zp = np.einsum("rkc,rc->rk", nb, zl[rc])
            pos[rc] = (zp >= 0).sum(axis=1)
        final = np.where((2 * pos - K >= 0)[:, None], zl, -zl)
        delta = (final - zsb).astype(np.float32)
        _mark(f"vote{b}")

        # boundary-tie rows (cnt != 128): the device mask summed extra points;
        # swap in the device-exported exact top-K-set sum.
        cntrows_l = np.nonzero(cntd != 0)[0]
        cntrows = rows[cntrows_l]
        for r in cntrows:
            sl = hmap[r]
            if sl >= 0:
                Sg[b][r] = hS_b[sl]

        # propagate corrections: row r is affected iff d2(r, m) <= r2k_r
        # BY DEVICE ARITHMETIC. Fast sgemm for the bulk test; pairs within a
        # narrow band of the threshold are re-decided with a bitwise mirror
        # of the device's elementwise d2 (same products, same association).
        big = np.abs(delta).max(axis=1) > 1e-3
        cols = rows[big]
        if cols.size:
            global _corr_buf, _corr_f32
            if _corr_buf is None:
                _corr_buf = np.empty((FMAX, N), np.float32)
                _corr_f32 = np.empty((FMAX, N), np.float32)
            F = cols.size
            dl = delta[big]
            if F > FMAX:  # degrade gracefully: drop the smallest deltas
                keep = np.argsort(-np.abs(dl).max(axis=1))[:FMAX]
                cols = cols[keep]
                dl = dl[keep]
                F = FMAX
            vb = vertices[b]
            vm = (vb[cols] * np.float32(-2.0)).astype(np.float32)  # fold -2 into the gemm
            tr = n2[b] - r2g[b]  # prefold per-row terms: d2 - thr = -2dot + n2m + (n2r - thr)
            # fast approximate containment via sgemm in [F, N] layout
            # (contiguous rows), exact bitwise device mirror only for pairs
            # within a narrow band of the threshold
            D2 = _corr_buf[:F]
            np.matmul(vm, vb.T, out=D2)
            D2 += n2[b][cols][:, None]
            D2 += tr[None, :]  # now D2 holds margin = d2_approx - thr
            basef = _corr_f32[:F]
            np.less_equal(D2, 0.0, out=basef, casting="unsafe")
            np.abs(D2, out=D2)
            band = D2 <= 2e-4  # sgemm-vs-mirror drift is <~1e-5; 20x margin
            hot = np.nonzero(band.sum(axis=1))[0]  # flagged cols with band pairs
            if hot.size:
                ari_l, aci_l = [], []
                for c in hot:
                    rr = np.nonzero(band[c])[0]
                    ari_l.append(rr)
                    aci_l.append(np.full(rr.size, c, np.int64))
                ari = np.concatenate(ari_l)
                aci = np.concatenate(aci_l)
                va, vc = vb[ari], vb[cols[aci]]
                # exact mirror: (p0 + p1) + p2, then (sq_q - 2 dot) + sq_all
                dot = (va[:, 0] * vc[:, 0] + va[:, 1] * vc[:, 1]) + va[:, 2] * vc[:, 2]
                d2x = (n2[b][ari] - np.float32(2.0) * dot) + n2[b][cols[aci]]
                basef[aci, ari] = (d2x <= r2g[b][ari]).astype(np.float32)
            # boundary-tie rows got the exact exported sum: apply their
            # corrections from the exported index row instead
            if cntrows.size:
                basef[:, cntrows] = 0.0
                colpos = np.full(N, -1, np.int32)
                colpos[cols] = np.arange(cols.size)
                for r in cntrows:
                    sl = hmap[r]
                    if sl >= 0:
                        cps = colpos[hidx_b[sl]]
                        for cp in cps[cps >= 0]:
                            Sg[b][r] += dl[cp]
            Sg[b] += basef.T @ dl
        # sub-threshold deltas (aligned, ~1e-6) are dropped: their effect on a
        # 128-normal average is < 1e-8
        _mark(f"corr{b}")

    nrm = np.sqrt(np.einsum("bnc,bnc->bn", Sg, Sg))
    Sg /= nrm[:, :, None]
    if _DEBUG_T:
        t3 = time.perf_counter()
        print(
            f"[kernel] dispatch {(t1-t0)*1e3:.1f}ms  sync+pull {(t2-t1)*1e3:.1f}ms"
            f"  host-fix {(t3-t2)*1e3:.1f}ms  "
            + " ".join(f"{k}={v*1e3:.1f}" for k, v in _tmarks),
            flush=True,
        )
    return Sg


# revision 4
# speedup vs baseline: 1.1422x; 1.0162x over previous
"""AveragedNormals on 8 Trainium2 NeuronCores — gather-free single-sync design.

Tunnel model (measured): every host<->device sync costs a fixed ~40-85ms RTT
(network-dependent), pull bandwidth ~77MB/s, chained dispatches are free.
Device-side indirect gathers are the other big cost (~55ms for the baseline's
two [2048x128] gathers), while lax.top_k on [2048, 8192] is only ~6ms.

So this kernel removes ALL device gathers by exploiting the SHOT weight
structure: w_j = radius - d_j is >= 0 exactly for the 128 nearest neighbors
and the weight of the 128th is exactly 0, so

  cov_n  = sum_j relu(radius_n - d_nj) x_nj x_nj^T   (x = v_j - q_n)
  vote_n = sum_j [d2_nj <= r2k_n] f(x_nj . z_n)
  S_n    = sum_j [d2_nj <= r2k_n] zsigned_j          (masked matmul)

over ALL 8192 points — identical term sets to the reference's gathered top-128
versions (only fp summation order differs, ~1e-7). Only the 128th-smallest
distance r2k per row is needed (top_k values; the index matrix is used solely
for the small hard-row export).

Correctness model (vs reference = top_k + LAPACK eigh + vote + gather-mean),
same as the baseline: ambiguous rows (vote ties in [0, 2*zeta], unstable
counts, weak eigengap, mask-count != 128) are exported compactly (cov6 + zs +
aux for <=448 rows/core) and fixed on host with np.linalg.eigh; corrections
delta_m = z_final - z_device propagate to every row whose neighborhood holds m
via a HOST-side distance matmul (vertices @ flipped^T vs pulled r2k) instead of
pulling per-row neighbor lists — cutting the pull from ~2.4MB to ~0.5MB.
"""

import functools
import os
import time

import jax
import jax.numpy as jnp
import numpy as np
from jax import lax

_DEBUG_T = bool(os.environ.get("AN_DEBUG_T"))
_DEBUG_FULL = bool(os.environ.get("AN_DEBUG_FULL"))
_tmarks = []

B = 2
N = 8192
K = 128
SPLIT = 4  # row-split per sample
NC = 8
ROWS = N // SPLIT  # 2048
EPS = 1e-12
FCAP = 512  # per-core flagged-row export capacity (tie rate is input-
# dependent: 5-10% observed across PRNG backends, theory ~14%; 512 covers 25%)
HARD = 48  # per-core exported hard-row (recount) capacity (measured max ~29)
HI = lax.Precision.HIGHEST
GROUPS = [[0, 1, 2, 3], [4, 5, 6, 7]]


def _smallest_evec_gap(cov):
    # cov: [R, 3, 3] symmetric. Unit eigenvector of the smallest eigenvalue
    # plus the relative gap (lam_mid - lam_min) / (lam_max - lam_min).
    a00 = cov[:, 0, 0]
    a01 = cov[:, 0, 1]
    a02 = cov[:, 0, 2]
    a11 = cov[:, 1, 1]
    a12 = cov[:, 1, 2]
    a22 = cov[:, 2, 2]

    q = (a00 + a11 + a22) / 3.0
    b00 = a00 - q
    b11 = a11 - q
    b22 = a22 - q
    p1 = a01 * a01 + a02 * a02 + a12 * a12
    p2 = b00 * b00 + b11 * b11 + b22 * b22 + 2.0 * p1
    p = jnp.sqrt(jnp.maximum(p2 / 6.0, 1e-30))
    detb = (
        b00 * (b11 * b22 - a12 * a12)
        - a01 * (a01 * b22 - a12 * a02)
        + a02 * (a01 * a12 - b11 * a02)
    )
    r = jnp.clip(detb / (2.0 * p * p * p), -1.0, 1.0)
    # acos via atan2 (mhlo.acos doesn't lower on the neuron backend)
    phi = jnp.arctan2(jnp.sqrt(jnp.maximum(1.0 - r * r, 0.0)), r) / 3.0
    lam_hi = q + 2.0 * p * jnp.cos(phi)
    lam = q + 2.0 * p * jnp.cos(phi + 2.0 * np.pi / 3.0)  # smallest
    lam_mid = 3.0 * q - lam_hi - lam
    spread = jnp.maximum(lam_hi - lam, 1e-30)
    gapr = (lam_mid - lam) / spread

    m00 = a00 - lam
    m11 = a11 - lam
    m22 = a22 - lam
    r0 = jnp.stack([m00, a01, a02], axis=-1)
    r1 = jnp.stack([a01, m11, a12], axis=-1)
    r2 = jnp.stack([a02, a12, m22], axis=-1)
    c01 = jnp.cross(r0, r1)
    c02 = jnp.cross(r0, r2)
    c12 = jnp.cross(r1, r2)
    n01 = jnp.sum(c01 * c01, axis=-1)
    n02 = jnp.sum(c02 * c02, axis=-1)
    n12 = jnp.sum(c12 * c12, axis=-1)
    best12 = (n12 >= n01) & (n12 >= n02)
    best02 = (n02 >= n01) & ~best12
    v = jnp.where(best12[:, None], c12, jnp.where(best02[:, None], c02, c01))
    nv = jnp.sqrt(jnp.maximum(jnp.sum(v * v, axis=-1, keepdims=True), 1e-30))
    v = v / nv

    # Two inverse-iteration refinements (Rayleigh quotient + adjugate solve):
    # the closed-form z is only ~1e-3 accurate; the vote is decided by
    # near-zero neighbor projections, so z must match eigh to ~1e-6.
    eps_reg = 1e-7 * jnp.maximum(jnp.abs(q), p)
    for _ in range(2):
        lam_r = (
            v[:, 0] * (a00 * v[:, 0] + a01 * v[:, 1] + a02 * v[:, 2])
            + v[:, 1] * (a01 * v[:, 0] + a11 * v[:, 1] + a12 * v[:, 2])
            + v[:, 2] * (a02 * v[:, 0] + a12 * v[:, 1] + a22 * v[:, 2])
        )
        m00 = a00 - lam_r + eps_reg
        m11 = a11 - lam_r + eps_reg
        m22 = a22 - lam_r + eps_reg
        y0 = (
            (m11 * m22 - a12 * a12) * v[:, 0]
            + (a02 * a12 - a01 * m22) * v[:, 1]
            + (a01 * a12 - a02 * m11) * v[:, 2]
        )
        y1 = (
            (a02 * a12 - a01 * m22) * v[:, 0]
            + (m00 * m22 - a02 * a02) * v[:, 1]
            + (a01 * a02 - m00 * a12) * v[:, 2]
        )
        y2 = (
            (a01 * a12 - a02 * m11) * v[:, 0]
            + (a01 * a02 - m00 * a12) * v[:, 1]
            + (m00 * m11 - a01 * a01) * v[:, 2]
        )
        y = jnp.stack([y0, y1, y2], axis=-1)
        y = jnp.where(jnp.sum(y * v, axis=-1, keepdims=True) < 0, -y, y)
        ny = jnp.sqrt(jnp.maximum(jnp.sum(y * y, axis=-1, keepdims=True), 1e-38))
        v = y / ny
    return v, gapr


def _stage1_body(v_sh):
    # v_sh: [ROWS, 3] this core's query block; replicate the full cloud via a
    # psum over the sample's 4-way split axis of disjoint zero-padded blocks
    # (bit-identical, 4x less push)
    vq = v_sh
    row0 = lax.axis_index("x") * ROWS
    vfp = jnp.zeros((N, 3), jnp.float32)
    vfp = lax.dynamic_update_slice(vfp, v_sh, (row0, 0))
    v_full = lax.psum(vfp, "x")

    # d2 via ELEMENTWISE ops in a fixed association order, bitwise-reproducible
    # by numpy on the host: the host correction step must replicate the
    # device's neighborhood-membership test d2 <= r2k exactly, and a matmul's
    # accumulation order can't be reproduced host-side. Mirror this exact
    # op sequence (products, left-assoc sums) in kernel()'s containment test.
    a0 = v_full[:, 0]
    a1 = v_full[:, 1]
    a2 = v_full[:, 2]
    sq_all = (a0 * a0 + a1 * a1) + a2 * a2  # [N]
    q0 = vq[:, 0]
    q1 = vq[:, 1]
    q2 = vq[:, 2]
    sq_q = (q0 * q0 + q1 * q1) + q2 * q2  # [ROWS]
    dot = (q0[:, None] * a0[None, :] + q1[:, None] * a1[None, :]) + q2[:, None] * a2[None, :]
    d2 = (sq_q[:, None] - 2.0 * dot) + sq_all[None, :]  # [ROWS, N]
    # One materialization of d2 shared by top_k / mask / weights: XLA would
    # otherwise rematerialize it per consumer fusion with different rounding,
    # silently swapping boundary neighbors between the top-k set and the mask.
    d2 = lax.optimization_barrier(d2)

    # sqrt is monotone, so top-k of -d2 selects the same neighbor set as the
    # reference's top-k of -sqrt(max(d2, EPS)) (the EPS plateau only merges
    # the self-distance, which is in the set either way).
    neg2, idx = lax.top_k(-d2, K)
    r2k = -neg2[:, -1]  # [ROWS] squared distance of the 128th-nearest
    radius = jnp.sqrt(jnp.maximum(r2k, EPS))

    # gather-free masked reductions over all N points
    x0 = v_full[:, 0][None, :] - vq[:, 0][:, None]  # [ROWS, N]
    x1 = v_full[:, 1][None, :] - vq[:, 1][:, None]
    x2 = v_full[:, 2][None, :] - vq[:, 2][:, None]
    d = jnp.sqrt(jnp.maximum(d2, EPS))
    w = jnp.maximum(radius[:, None] - d, 0.0)  # SHOT weight; 0 outside top-K
    mask = (d2 <= r2k[:, None]).astype(jnp.float32)  # [ROWS, N]

    cnt = jnp.sum(mask, axis=-1)  # 128 unless d2 ties at the boundary
    wsum = jnp.sum(w, axis=-1)
    c00 = jnp.sum(w * x0 * x0, axis=-1)
    c11 = jnp.sum(w * x1 * x1, axis=-1)
    c22 = jnp.sum(w * x2 * x2, axis=-1)
    c01 = jnp.sum(w * x0 * x1, axis=-1)
    c02 = jnp.sum(w * x0 * x2, axis=-1)
    c12 = jnp.sum(w * x1 * x2, axis=-1)
    iw = 1.0 / wsum
    cov6 = jnp.stack([c00, c11, c22, c01, c02, c12], axis=-1) * iw[:, None]
    cov = jnp.stack(
        [
            jnp.stack([c00, c01, c02], axis=-1),
            jnp.stack([c01, c11, c12], axis=-1),
            jnp.stack([c02, c12, c22], axis=-1),
        ],
        axis=1,
    ) * iw[:, None, None]

    z0, gapr = _smallest_evec_gap(cov)  # [ROWS, 3], [ROWS]

    # SHOT sign vote with the device eigenvector, masked over all N
    zp = x0 * z0[:, 0][:, None] + x1 * z0[:, 1][:, None] + x2 * z0[:, 2][:, None]
    posc = jnp.sum(mask * (zp >= 0), axis=-1).astype(jnp.int32)
    zeta = jnp.sum(mask * (zp == 0), axis=-1).astype(jnp.int32)
    abszp = jnp.where((mask > 0) & (zp != 0), jnp.abs(zp), jnp.float32(np.inf))
    minabs = jnp.min(abszp, axis=-1)
    margin = 2 * posc - K
    s = jnp.where(margin >= 0, 1.0, -1.0).astype(jnp.float32)
    zs = s[:, None] * z0  # vote-oriented device normal

    # ambiguity flags (host fixes these rows with LAPACK eigh)
    f_tie = (margin >= 0) & (margin <= 2 * zeta)  # LAPACK sign decides
    f_zp = (
        (minabs < 3e-5 * radius) & (margin >= -4) & (margin <= 2 * zeta + 4)
    ) | (zeta > 1)  # counts unstable near a boundary (z0 error ~1e-6)
    f_gap = gapr < 3e-3  # device eigenvector unreliable
    f_cnt = cnt != 128.0  # boundary tie: mask picked up extras
    recount = f_zp | f_gap | f_cnt
    flag = f_tie.astype(jnp.int32) + 2 * recount.astype(jnp.int32)

    # replicate signed normals across the sample's 4-core group
    zfull = jnp.zeros((N, 3), jnp.float32)
    zfull = lax.dynamic_update_slice(zfull, zs, (row0, 0))
    zfull = lax.psum(zfull, "x")

    # neighbor-average numerator via masked matmul (no gather)
    S = lax.dot_general(mask, zfull, (((1,), (0,)), ((), ())), precision=HI)

    # compacted flagged-row export (f32 scores: TopK rejects integer inputs)
    fsc = (flag > 0).astype(jnp.float32) * 100000.0 + jnp.arange(
        ROWS, dtype=jnp.float32
    )
    fval, frow = lax.top_k(fsc, FCAP)
    frows = jnp.where(fval >= 100000.0, frow, -1).astype(jnp.int16)
    fcov6 = jnp.take(cov6, frow, axis=0)  # [FCAP, 6]
    fzs = jnp.take(zs, frow, axis=0)  # [FCAP, 3]
    faux = jnp.stack(
        [
            jnp.take(margin, frow) // 2,  # margin even: store margin/2
            jnp.minimum(jnp.take(zeta, frow), 127),
            jnp.take(flag, frow),
            jnp.clip(jnp.take(cnt.astype(jnp.int32), frow) - K, -8, 8),
        ],
        axis=-1,
    ).astype(jnp.int8)

    # export top-K index rows for rows needing a host vote recount
    hsc = recount.astype(jnp.float32) * 100000.0 + jnp.arange(
        ROWS, dtype=jnp.float32
    )
    hval, hrow = lax.top_k(hsc, HARD)
    hard_rows = jnp.where(hval >= 100000.0, hrow, -1).astype(jnp.int16)
    hard_idx = jnp.take(idx, hrow, axis=0).astype(jnp.int16)  # [HARD, K]
    # exact top-K-set sum for hard rows (tiny gather): boundary-tie rows'
    # masked S picked up extras; the host swaps in this exact sum instead
    hard_S = jnp.take(zfull, jnp.take(idx, hrow, axis=0).reshape(-1), axis=0)
    hard_S = jnp.sum(hard_S.reshape(HARD, K, 3), axis=1)  # [HARD, 3]

    # pack into few arrays: per-array-per-shard pull overhead is measurable
    main = jnp.concatenate([S, r2k[:, None]], axis=1)  # [ROWS, 4] f32
    fblk = jnp.concatenate(
        [
            fcov6,
            fzs,
            frows.astype(jnp.float32)[:, None],  # int16 values, exact in f32
            faux.astype(jnp.float32),
        ],
        axis=1,
    )  # [FCAP, 14] f32
    hblk = jnp.concatenate([hard_rows[:, None], hard_idx], axis=1)  # [HARD, 129] i16
    if _DEBUG_FULL:
        return main, fblk, hblk, hard_S, zs, jnp.stack([margin, zeta, flag], -1), cov6, gapr[:, None], minabs[:, None], idx.astype(jnp.int16), cnt[:, None]
    return main, fblk, hblk, hard_S


_corr_buf = None  # preallocated [FMAX, N] f32 scratch for the host correction
_corr_f32 = None
FMAX = 1024  # max big-delta columns per sample (observed up to ~430)
_mesh = None
_stage1 = None


def _build():
    global _mesh, _stage1
    from jax.experimental.shard_map import shard_map
    from jax.sharding import Mesh, PartitionSpec as P

    devs = np.array(jax.devices()[:NC]).reshape(B, SPLIT)
    _mesh = Mesh(devs, ("b", "x"))

    def body(v_blk):
        # v_blk: [1, 1, ROWS, 3] block of [B, SPLIT, ROWS, 3]
        outs = _stage1_body(v_blk[0, 0])
        return tuple(o[None, None] for o in outs)

    nout = 11 if _DEBUG_FULL else 4
    sm = shard_map(
        body,
        mesh=_mesh,
        in_specs=(P("b", "x", None, None),),
        out_specs=tuple(P("b", "x", None, None) for _ in range(nout)),
    )
    _stage1 = jax.jit(sm)


def kernel(vertices: np.ndarray) -> np.ndarray:
    vertices = np.asarray(vertices, dtype=np.float32)
    assert vertices.shape == (B, N, 3)
    v_sh = vertices.reshape(B, SPLIT, ROWS, 3)

    t0 = time.perf_counter()
    if _stage1 is None:
        _build()
    outs = _stage1(jnp.asarray(v_sh))
    t1 = time.perf_counter()
    pulled = jax.device_get(outs)
    t2 = time.perf_counter()
    global _last_pull
    _last_pull = pulled
    main = pulled[0].reshape(NC, ROWS, 4)
    fblk = pulled[1].reshape(NC, FCAP, 14)
    hblk = pulled[2].reshape(NC, HARD, 129)
    hard_S = pulled[3].reshape(NC, HARD, 3)
    S = main[:, :, :3]
    r2k = main[:, :, 3]
    fcov6 = fblk[:, :, 0:6]
    fzs = fblk[:, :, 6:9]
    frows_f = fblk[:, :, 9]
    faux_f = fblk[:, :, 10:14]
    hard_rows = hblk[:, :, 0]
    hard_idx = hblk[:, :, 1:]

    _tmarks.clear()
    tp = time.perf_counter()

    def _mark(name):
        nonlocal tp
        now = time.perf_counter()
        _tmarks.append((name, now - tp))
        tp = now

    # core c -> sample c//4, rows [(c%4)*ROWS, ...): plain reshape restores [B,N]
    Sg = np.array(S.reshape(B, N, 3))  # writable copy
    r2g = r2k.reshape(B, N)
    # n2 mirrors the device's sq_all association order exactly
    n2 = (vertices[..., 0] * vertices[..., 0] + vertices[..., 1] * vertices[..., 1]) + vertices[..., 2] * vertices[..., 2]

    # hard-row maps (vectorized): per-core flagged slots sort first
    nhard = (hard_rows >= 0).sum(axis=1)  # [NC]
    _mark("unpack")

    for b in range(B):
        # gather this sample's flagged-row export across its 4 cores
        rows_l, cov_l, zs_l, aux_l = [], [], [], []
        hmap = np.full(N, -1, np.int32)
        hidx_l, hS_l = [], []
        hn = 0
        for c in range(b * SPLIT, (b + 1) * SPLIT):
            nv = int((frows_f[c] >= 0).sum())
            off = (c % SPLIT) * ROWS
            rows_l.append(frows_f[c, :nv].astype(np.int32) + off)
            cov_l.append(fcov6[c, :nv])
            zs_l.append(fzs[c, :nv])
            aux_l.append(faux_f[c, :nv])
            nh = int(nhard[c])
            hmap[hard_rows[c, :nh].astype(np.int32) + off] = hn + np.arange(nh)
            hidx_l.append(hard_idx[c, :nh].astype(np.int32))
            hS_l.append(hard_S[c, :nh])
            hn += nh
        rows = np.concatenate(rows_l)
        if _DEBUG_T:
            nv_pc = [int((frows_f[c] >= 0).sum()) for c in range(b * SPLIT, (b + 1) * SPLIT)]
            print(f"[kernel] sample {b}: flagged/core {nv_pc} (cap {FCAP}), hard {hn} (cap {HARD}/core)", flush=True)
        if rows.size == 0:
            continue
        cov6b = np.concatenate(cov_l).astype(np.float32)
        zsb = np.concatenate(zs_l).astype(np.float32)
        auxb = np.concatenate(aux_l).astype(np.int32)
        hidx_b = np.concatenate(hidx_l) if hn else np.zeros((0, K), np.int32)
        hS_b = np.concatenate(hS_l) if hn else np.zeros((0, 3), np.float32)
        mg = 2 * auxb[:, 0]
        zeta = auxb[:, 1]
        flag = auxb[:, 2]
        cntd = auxb[:, 3]  # cnt - 128 (boundary-tie rows have cntd != 0)
        _mark(f"gather{b}")

        covg = np.empty((rows.size, 3, 3), np.float32)
        covg[:, 0, 0] = cov6b[:, 0]
        covg[:, 1, 1] = cov6b[:, 1]
        covg[:, 2, 2] = cov6b[:, 2]
        covg[:, 0, 1] = covg[:, 1, 0] = cov6b[:, 3]
        covg[:, 0, 2] = covg[:, 2, 0] = cov6b[:, 4]
        covg[:, 1, 2] = covg[:, 2, 1] = cov6b[:, 5]
        # LAPACK eigh only on ambiguous rows: its sign convention is the spec
        _, vecs = np.linalg.eigh(covg)
        zl = np.ascontiguousarray(vecs[:, :, 0])  # [R, 3]
        _mark(f"eigh{b}")

        z0 = np.where(mg >= 0, 1.0, -1.0).astype(np.float32)[:, None] * zsb
        # remap device counts to the LAPACK orientation: pos(-z) = neg(z) + zeta
        sigma = np.einsum("rc,rc->r", zl, z0)
        pos = np.where(sigma >= 0, (mg + K) // 2, (K - mg) // 2 + zeta)
        # rows needing a true recount (unstable counts / unreliable device vec)
        rc = np.nonzero((flag >= 2) & (hmap[rows] >= 0))[0]
        if rc.size:
            slots = hmap[rows[rc]]
            nb = vertices[b][hidx_b[slots]] - vertices[b][rows[rc], None, :]
            zp = np.einsum("rkc,rc->rk", nb, zl[rc])
            pos[rc] = (zp >= 0).sum(axis=1)
        final = np.where((2 * pos - K >= 0)[:, None], zl, -zl)
        delta = (final - zsb).astype(np.float32)
        _mark(f"vote{b}")

        # boundary-tie rows (cnt != 128): the device mask summed extra points;
        # swap in the device-exported exact top-K-set sum.
        cntrows_l = np.nonzero(cntd != 0)[0]
        cntrows = rows[cntrows_l]
        for r in cntrows:
            sl = hmap[r]
            if sl >= 0:
                Sg[b][r] = hS_b[sl]

        # propagate corrections: row r is affected iff d2(r, m) <= r2k_r
        # BY DEVICE ARITHMETIC. Fast sgemm for the bulk test; pairs within a
        # narrow band of the threshold are re-decided with a bitwise mirror
        # of the device's elementwise d2 (same products, same association).
        big = np.abs(delta).max(axis=1) > 1e-3
        cols = rows[big]
        if cols.size:
            global _corr_buf, _corr_f32
            if _corr_buf is None:
                _corr_buf = np.empty((FMAX, N), np.float32)
                _corr_f32 = np.empty((FMAX, N), np.float32)
            F = cols.size
            dl = delta[big]
            if F > FMAX:  # degrade gracefully: drop the smallest deltas
                keep = np.argsort(-np.abs(dl).max(axis=1))[:FMAX]
                cols = cols[keep]
                dl = dl[keep]
                F = FMAX
            vb = vertices[b]
            vm = (vb[cols] * np.float32(-2.0)).astype(np.float32)  # fold -2 into the gemm
            tr = n2[b] - r2g[b]  # prefold per-row terms: d2 - thr = -2dot + n2m + (n2r - thr)
            # fast approximate containment via sgemm in [F, N] layout
            # (contiguous rows), exact bitwise device mirror only for pairs
            # within a narrow band of the threshold
            D2 = _corr_buf[:F]
            np.matmul(vm, vb.T, out=D2)
            D2 += n2[b][cols][:, None]
            D2 += tr[None, :]  # now D2 holds margin = d2_approx - thr
            basef = _corr_f32[:F]
            np.less_equal(D2, 0.0, out=basef, casting="unsafe")
            np.abs(D2, out=D2)
            band = D2 <= 2e-4  # sgemm-vs-mirror drift is <~1e-5; 20x margin
            hot = np.nonzero(band.sum(axis=1))[0]  # flagged cols with band pairs
            if hot.size:
                ari_l, aci_l = [], []
                for c in hot:
                    rr = np.nonzero(band[c])[0]
                    ari_l.append(rr)
                    aci_l.append(np.full(rr.size, c, np.int64))
                ari = np.concatenate(ari_l)
                aci = np.concatenate(aci_l)
                va, vc = vb[ari], vb[cols[aci]]
                # exact mirror: (p0 + p1) + p2, then (sq_q - 2 dot) + sq_all
                dot = (va[:, 0] * vc[:, 0] + va[:, 1] * vc[:, 1]) + va[:, 2] * vc[:, 2]
                d2x = (n2[b][ari] - np.float32(2.0) * dot) + n2[b][cols[aci]]
                basef[aci, ari] = (d2x <= r2g[b][ari]).astype(np.float32)
            # boundary-tie rows got the exact exported sum: apply their
            # corrections from the exported index row instead
            if cntrows.size:
                basef[:, cntrows] = 0.0
                colpos = np.full(N, -1, np.int32)
                colpos[cols] = np.arange(cols.size)
                for r in cntrows:
                    sl = hmap[r]
                    if sl >= 0:
                        cps = colpos[hidx_b[sl]]
                        for cp in cps[cps >= 0]:
                            Sg[b][r] += dl[cp]
            Sg[b] += basef.T @ dl
        # sub-threshold deltas (aligned, ~1e-6) are dropped: their effect on a
        # 128-normal average is < 1e-8
        _mark(f"corr{b}")

    nrm = np.sqrt(np.einsum("bnc,bnc->bn", Sg, Sg))
    Sg /= nrm[:, :, None]
    if _DEBUG_T:
        t3 = time.perf_counter()
        print(
            f"[kernel] dispatch {(t1-t0)*1e3:.1f}ms  sync+pull {(t2-t1)*1e3:.1f}ms"
            f"  host-fix {(t3-t2)*1e3:.1f}ms  "
            + " ".join(f"{k}={v*1e3:.1f}" for k, v in _tmarks),
            flush=True,
        )
    return Sg


# revision 5
# speedup vs baseline: 1.1655x; 1.0204x over previous
"""AveragedNormals on 8 Trainium2 NeuronCores — gather-free single-sync design.

Tunnel model (measured): every host<->device sync costs a fixed ~40-85ms RTT
(network-dependent), pull bandwidth ~77MB/s, chained dispatches are free.
Device-side indirect gathers are the other big cost (~55ms for the baseline's
two [2048x128] gathers), while lax.top_k on [2048, 8192] is only ~6ms.

So this kernel removes ALL device gathers by exploiting the SHOT weight
structure: w_j = radius - d_j is >= 0 exactly for the 128 nearest neighbors
and the weight of the 128th is exactly 0, so

  cov_n  = sum_j relu(radius_n - d_nj) x_nj x_nj^T   (x = v_j - q_n)
  vote_n = sum_j [d2_nj <= r2k_n] f(x_nj . z_n)
  S_n    = sum_j [d2_nj <= r2k_n] zsigned_j          (masked matmul)

over ALL 8192 points — identical term sets to the reference's gathered top-128
versions (only fp summation order differs, ~1e-7). Only the 128th-smallest
distance r2k per row is needed (top_k values; the index matrix is used solely
for the small hard-row export).

Correctness model (vs reference = top_k + LAPACK eigh + vote + gather-mean),
same as the baseline: ambiguous rows (vote ties in [0, 2*zeta], unstable
counts, weak eigengap, mask-count != 128) are exported compactly (cov6 + zs +
aux for <=448 rows/core) and fixed on host with np.linalg.eigh; corrections
delta_m = z_final - z_device propagate to every row whose neighborhood holds m
via a HOST-side distance matmul (vertices @ flipped^T vs pulled r2k) instead of
pulling per-row neighbor lists — cutting the pull from ~2.4MB to ~0.5MB.
"""

import functools
import os
import time

import jax
import jax.numpy as jnp
import numpy as np
from jax import lax

_DEBUG_T = bool(os.environ.get("AN_DEBUG_T"))
_DEBUG_FULL = bool(os.environ.get("AN_DEBUG_FULL"))
_tmarks = []

B = 2
N = 8192
K = 128
SPLIT = 4  # row-split per sample
NC = 8
ROWS = N // SPLIT  # 2048
EPS = 1e-12
FCAP = 512  # per-core flagged-row export capacity (tie rate is input-
# dependent: 5-10% observed across PRNG backends, theory ~14%; 512 covers 25%)
HARD = 48  # per-core exported hard-row (recount) capacity (measured max ~29)
HI = lax.Precision.HIGHEST
GROUPS = [[0, 1, 2, 3], [4, 5, 6, 7]]


def _smallest_evec_gap(cov):
    # cov: [R, 3, 3] symmetric. Unit eigenvector of the smallest eigenvalue
    # plus the relative gap (lam_mid - lam_min) / (lam_max - lam_min).
    a00 = cov[:, 0, 0]
    a01 = cov[:, 0, 1]
    a02 = cov[:, 0, 2]
    a11 = cov[:, 1, 1]
    a12 = cov[:, 1, 2]
    a22 = cov[:, 2, 2]

    q = (a00 + a11 + a22) / 3.0
    b00 = a00 - q
    b11 = a11 - q
    b22 = a22 - q
    p1 = a01 * a01 + a02 * a02 + a12 * a12
    p2 = b00 * b00 + b11 * b11 + b22 * b22 + 2.0 * p1
    p = jnp.sqrt(jnp.maximum(p2 / 6.0, 1e-30))
    detb = (
        b00 * (b11 * b22 - a12 * a12)
        - a01 * (a01 * b22 - a12 * a02)
        + a02 * (a01 * a12 - b11 * a02)
    )
    r = jnp.clip(detb / (2.0 * p * p * p), -1.0, 1.0)
    # acos via atan2 (mhlo.acos doesn't lower on the neuron backend)
    phi = jnp.arctan2(jnp.sqrt(jnp.maximum(1.0 - r * r, 0.0)), r) / 3.0
    lam_hi = q + 2.0 * p * jnp.cos(phi)
    lam = q + 2.0 * p * jnp.cos(phi + 2.0 * np.pi / 3.0)  # smallest
    lam_mid = 3.0 * q - lam_hi - lam
    spread = jnp.maximum(lam_hi - lam, 1e-30)
    gapr = (lam_mid - lam) / spread

    m00 = a00 - lam
    m11 = a11 - lam
    m22 = a22 - lam
    r0 = jnp.stack([m00, a01, a02], axis=-1)
    r1 = jnp.stack([a01, m11, a12], axis=-1)
    r2 = jnp.stack([a02, a12, m22], axis=-1)
    c01 = jnp.cross(r0, r1)
    c02 = jnp.cross(r0, r2)
    c12 = jnp.cross(r1, r2)
    n01 = jnp.sum(c01 * c01, axis=-1)
    n02 = jnp.sum(c02 * c02, axis=-1)
    n12 = jnp.sum(c12 * c12, axis=-1)
    best12 = (n12 >= n01) & (n12 >= n02)
    best02 = (n02 >= n01) & ~best12
    v = jnp.where(best12[:, None], c12, jnp.where(best02[:, None], c02, c01))
    nv = jnp.sqrt(jnp.maximum(jnp.sum(v * v, axis=-1, keepdims=True), 1e-30))
    v = v / nv

    # Two inverse-iteration refinements (Rayleigh quotient + adjugate solve):
    # the closed-form z is only ~1e-3 accurate; the vote is decided by
    # near-zero neighbor projections, so z must match eigh to ~1e-6.
    eps_reg = 1e-7 * jnp.maximum(jnp.abs(q), p)
    for _ in range(2):
        lam_r = (
            v[:, 0] * (a00 * v[:, 0] + a01 * v[:, 1] + a02 * v[:, 2])
            + v[:, 1] * (a01 * v[:, 0] + a11 * v[:, 1] + a12 * v[:, 2])
            + v[:, 2] * (a02 * v[:, 0] + a12 * v[:, 1] + a22 * v[:, 2])
        )
        m00 = a00 - lam_r + eps_reg
        m11 = a11 - lam_r + eps_reg
        m22 = a22 - lam_r + eps_reg
        y0 = (
            (m11 * m22 - a12 * a12) * v[:, 0]
            + (a02 * a12 - a01 * m22) * v[:, 1]
            + (a01 * a12 - a02 * m11) * v[:, 2]
        )
        y1 = (
            (a02 * a12 - a01 * m22) * v[:, 0]
            + (m00 * m22 - a02 * a02) * v[:, 1]
            + (a01 * a02 - m00 * a12) * v[:, 2]
        )
        y2 = (
            (a01 * a12 - a02 * m11) * v[:, 0]
            + (a01 * a02 - m00 * a12) * v[:, 1]
            + (m00 * m11 - a01 * a01) * v[:, 2]
        )
        y = jnp.stack([y0, y1, y2], axis=-1)
        y = jnp.where(jnp.sum(y * v, axis=-1, keepdims=True) < 0, -y, y)
        ny = jnp.sqrt(jnp.maximum(jnp.sum(y * y, axis=-1, keepdims=True), 1e-38))
        v = y / ny
    return v, gapr


def _stage1_body(v_sh):
    # v_sh: [ROWS, 3] this core's query block; replicate the full cloud via a
    # psum over the sample's 4-way split axis of disjoint zero-padded blocks
    # (bit-identical, 4x less push)
    vq = v_sh
    row0 = lax.axis_index("x") * ROWS
    vfp = jnp.zeros((N, 3), jnp.float32)
    vfp = lax.dynamic_update_slice(vfp, v_sh, (row0, 0))
    v_full = lax.psum(vfp, "x")

    # d2 via ELEMENTWISE ops in a fixed association order, bitwise-reproducible
    # by numpy on the host: the host correction step must replicate the
    # device's neighborhood-membership test d2 <= r2k exactly, and a matmul's
    # accumulation order can't be reproduced host-side. Mirror this exact
    # op sequence (products, left-assoc sums) in kernel()'s containment test.
    a0 = v_full[:, 0]
    a1 = v_full[:, 1]
    a2 = v_full[:, 2]
    sq_all = (a0 * a0 + a1 * a1) + a2 * a2  # [N]
    q0 = vq[:, 0]
    q1 = vq[:, 1]
    q2 = vq[:, 2]
    sq_q = (q0 * q0 + q1 * q1) + q2 * q2  # [ROWS]
    dot = (q0[:, None] * a0[None, :] + q1[:, None] * a1[None, :]) + q2[:, None] * a2[None, :]
    d2 = (sq_q[:, None] - 2.0 * dot) + sq_all[None, :]  # [ROWS, N]
    # One materialization of d2 shared by top_k / mask / weights: XLA would
    # otherwise rematerialize it per consumer fusion with different rounding,
    # silently swapping boundary neighbors between the top-k set and the mask.
    d2 = lax.optimization_barrier(d2)

    # sqrt is monotone, so top-k of -d2 selects the same neighbor set as the
    # reference's top-k of -sqrt(max(d2, EPS)) (the EPS plateau only merges
    # the self-distance, which is in the set either way).
    neg2, idx = lax.top_k(-d2, K)
    r2k = -neg2[:, -1]  # [ROWS] squared distance of the 128th-nearest
    radius = jnp.sqrt(jnp.maximum(r2k, EPS))

    # gather-free masked reductions over all N points
    x0 = v_full[:, 0][None, :] - vq[:, 0][:, None]  # [ROWS, N]
    x1 = v_full[:, 1][None, :] - vq[:, 1][:, None]
    x2 = v_full[:, 2][None, :] - vq[:, 2][:, None]
    d = jnp.sqrt(jnp.maximum(d2, EPS))
    w = jnp.maximum(radius[:, None] - d, 0.0)  # SHOT weight; 0 outside top-K
    mask = (d2 <= r2k[:, None]).astype(jnp.float32)  # [ROWS, N]

    cnt = jnp.sum(mask, axis=-1)  # 128 unless d2 ties at the boundary
    wsum = jnp.sum(w, axis=-1)
    c00 = jnp.sum(w * x0 * x0, axis=-1)
    c11 = jnp.sum(w * x1 * x1, axis=-1)
    c22 = jnp.sum(w * x2 * x2, axis=-1)
    c01 = jnp.sum(w * x0 * x1, axis=-1)
    c02 = jnp.sum(w * x0 * x2, axis=-1)
    c12 = jnp.sum(w * x1 * x2, axis=-1)
    iw = 1.0 / wsum
    cov6 = jnp.stack([c00, c11, c22, c01, c02, c12], axis=-1) * iw[:, None]
    cov = jnp.stack(
        [
            jnp.stack([c00, c01, c02], axis=-1),
            jnp.stack([c01, c11, c12], axis=-1),
            jnp.stack([c02, c12, c22], axis=-1),
        ],
        axis=1,
    ) * iw[:, None, None]

    z0, gapr = _smallest_evec_gap(cov)  # [ROWS, 3], [ROWS]

    # SHOT sign vote with the device eigenvector, masked over all N
    zp = x0 * z0[:, 0][:, None] + x1 * z0[:, 1][:, None] + x2 * z0[:, 2][:, None]
    posc = jnp.sum(mask * (zp >= 0), axis=-1).astype(jnp.int32)
    zeta = jnp.sum(mask * (zp == 0), axis=-1).astype(jnp.int32)
    abszp = jnp.where((mask > 0) & (zp != 0), jnp.abs(zp), jnp.float32(np.inf))
    minabs = jnp.min(abszp, axis=-1)
    margin = 2 * posc - K
    s = jnp.where(margin >= 0, 1.0, -1.0).astype(jnp.float32)
    zs = s[:, None] * z0  # vote-oriented device normal

    # ambiguity flags (host fixes these rows with LAPACK eigh)
    f_tie = (margin >= 0) & (margin <= 2 * zeta)  # LAPACK sign decides
    f_zp = (
        (minabs < 3e-5 * radius) & (margin >= -4) & (margin <= 2 * zeta + 4)
    ) | (zeta > 1)  # counts unstable near a boundary (z0 error ~1e-6)
    f_gap = gapr < 3e-3  # device eigenvector unreliable
    f_cnt = cnt != 128.0  # boundary tie: mask picked up extras
    recount = f_zp | f_gap | f_cnt
    flag = f_tie.astype(jnp.int32) + 2 * recount.astype(jnp.int32)

    # replicate signed normals across the sample's 4-core group
    zfull = jnp.zeros((N, 3), jnp.float32)
    zfull = lax.dynamic_update_slice(zfull, zs, (row0, 0))
    zfull = lax.psum(zfull, "x")

    # neighbor-average numerator via masked matmul (no gather)
    S = lax.dot_general(mask, zfull, (((1,), (0,)), ((), ())), precision=HI)

    # compacted flagged-row export (f32 scores: TopK rejects integer inputs)
    fsc = (flag > 0).astype(jnp.float32) * 100000.0 + jnp.arange(
        ROWS, dtype=jnp.float32
    )
    fval, frow = lax.top_k(fsc, FCAP)
    frows = jnp.where(fval >= 100000.0, frow, -1).astype(jnp.int16)
    fcov6 = jnp.take(cov6, frow, axis=0)  # [FCAP, 6]
    fzs = jnp.take(zs, frow, axis=0)  # [FCAP, 3]
    faux = jnp.stack(
        [
            jnp.take(margin, frow) // 2,  # margin even: store margin/2
            jnp.minimum(jnp.take(zeta, frow), 127),
            jnp.take(flag, frow),
            jnp.clip(jnp.take(cnt.astype(jnp.int32), frow) - K, -8, 8),
        ],
        axis=-1,
    ).astype(jnp.int8)

    # export top-K index rows for rows needing a host vote recount
    hsc = recount.astype(jnp.float32) * 100000.0 + jnp.arange(
        ROWS, dtype=jnp.float32
    )
    hval, hrow = lax.top_k(hsc, HARD)
    hard_rows = jnp.where(hval >= 100000.0, hrow, -1).astype(jnp.int16)
    hard_idx = jnp.take(idx, hrow, axis=0).astype(jnp.int16)  # [HARD, K]
    # exact top-K-set sum for hard rows (tiny gather): boundary-tie rows'
    # masked S picked up extras; the host swaps in this exact sum instead
    hard_S = jnp.take(zfull, jnp.take(idx, hrow, axis=0).reshape(-1), axis=0)
    hard_S = jnp.sum(hard_S.reshape(HARD, K, 3), axis=1)  # [HARD, 3]

    # pack into few arrays: per-array-per-shard pull overhead is measurable
    main = jnp.concatenate([S, r2k[:, None]], axis=1)  # [ROWS, 4] f32
    fblk = jnp.concatenate(
        [
            fcov6,
            fzs,
            frows.astype(jnp.float32)[:, None],  # int16 values, exact in f32
            faux.astype(jnp.float32),
        ],
        axis=1,
    )  # [FCAP, 14] f32
    hblk = jnp.concatenate([hard_rows[:, None], hard_idx], axis=1)  # [HARD, 129] i16
    if _DEBUG_FULL:
        return main, fblk, hblk, hard_S, zs, jnp.stack([margin, zeta, flag], -1), cov6, gapr[:, None], minabs[:, None], idx.astype(jnp.int16), cnt[:, None]
    return main, fblk, hblk, hard_S


_corr_buf = None  # preallocated [FMAX, N] f32 scratch for the host correction
_corr_f32 = None
FMAX = 1024  # max big-delta columns per sample (observed up to ~430)
_mesh = None
_stage1 = None


def _build():
    global _mesh, _stage1
    from jax.experimental.shard_map import shard_map
    from jax.sharding import Mesh, PartitionSpec as P

    devs = np.array(jax.devices()[:NC]).reshape(B, SPLIT)
    _mesh = Mesh(devs, ("b", "x"))

    def body(v_blk):
        # v_blk: [1, 1, ROWS, 3] block of [B, SPLIT, ROWS, 3]
        outs = _stage1_body(v_blk[0, 0])
        return tuple(o[None, None] for o in outs)

    nout = 11 if _DEBUG_FULL else 4
    sm = shard_map(
        body,
        mesh=_mesh,
        in_specs=(P("b", "x", None, None),),
        out_specs=tuple(P("b", "x", None, None) for _ in range(nout)),
    )
    _stage1 = jax.jit(sm)


def kernel(vertices: np.ndarray) -> np.ndarray:
    vertices = np.asarray(vertices, dtype=np.float32)
    assert vertices.shape == (B, N, 3)
    v_sh = vertices.reshape(B, SPLIT, ROWS, 3)

    t0 = time.perf_counter()
    if _stage1 is None:
        _build()
    outs = _stage1(jnp.asarray(v_sh))
    t1 = time.perf_counter()
    pulled = jax.device_get(outs)
    t2 = time.perf_counter()
    global _last_pull
    _last_pull = pulled
    main = pulled[0].reshape(NC, ROWS, 4)
    fblk = pulled[1].reshape(NC, FCAP, 14)
    hblk = pulled[2].reshape(NC, HARD, 129)
    hard_S = pulled[3].reshape(NC, HARD, 3)
    S = main[:, :, :3]
    r2k = main[:, :, 3]
    fcov6 = fblk[:, :, 0:6]
    fzs = fblk[:, :, 6:9]
    frows_f = fblk[:, :, 9]
    faux_f = fblk[:, :, 10:14]
    hard_rows = hblk[:, :, 0]
    hard_idx = hblk[:, :, 1:]

    _tmarks.clear()
    tp = time.perf_counter()

    def _mark(name):
        nonlocal tp
        now = time.perf_counter()
        _tmarks.append((name, now - tp))
        tp = now

    # core c -> sample c//4, rows [(c%4)*ROWS, ...): plain reshape restores [B,N]
    Sg = np.array(S.reshape(B, N, 3))  # writable copy
    r2g = r2k.reshape(B, N)
    # n2 mirrors the device's sq_all association order exactly
    n2 = (vertices[..., 0] * vertices[..., 0] + vertices[..., 1] * vertices[..., 1]) + vertices[..., 2] * vertices[..., 2]

    # hard-row maps (vectorized): per-core flagged slots sort first
    nhard = (hard_rows >= 0).sum(axis=1)  # [NC]
    _mark("unpack")

    # phase 1: unpack per-sample flagged exports
    samples = []
    for b in range(B):
        rows_l, cov_l, zs_l, aux_l = [], [], [], []
        hmap = np.full(N, -1, np.int32)
        hidx_l, hS_l = [], []
        hn = 0
        for c in range(b * SPLIT, (b + 1) * SPLIT):
            nv = int((frows_f[c] >= 0).sum())
            off = (c % SPLIT) * ROWS
            rows_l.append(frows_f[c, :nv].astype(np.int32) + off)
            cov_l.append(fcov6[c, :nv])
            zs_l.append(fzs[c, :nv])
            aux_l.append(faux_f[c, :nv])
            nh = int(nhard[c])
            hmap[hard_rows[c, :nh].astype(np.int32) + off] = hn + np.arange(nh)
            hidx_l.append(hard_idx[c, :nh].astype(np.int32))
            hS_l.append(hard_S[c, :nh])
            hn += nh
        if _DEBUG_T:
            nv_pc = [int((frows_f[c] >= 0).sum()) for c in range(b * SPLIT, (b + 1) * SPLIT)]
            print(f"[kernel] sample {b}: flagged/core {nv_pc} (cap {FCAP}), hard {hn} (cap {HARD}/core)", flush=True)
        samples.append(
            (
                np.concatenate(rows_l),
                np.concatenate(cov_l).astype(np.float32),
                np.concatenate(zs_l).astype(np.float32),
                np.concatenate(aux_l).astype(np.int32),
                np.concatenate(hidx_l) if hn else np.zeros((0, K), np.int32),
                np.concatenate(hS_l) if hn else np.zeros((0, 3), np.float32),
                hmap,
            )
        )
    _mark("gather")

    # one merged LAPACK eigh across both samples: its sign convention is the spec
    covall = np.concatenate([s[1] for s in samples], axis=0)
    covg = np.empty((covall.shape[0], 3, 3), np.float32)
    covg[:, 0, 0] = covall[:, 0]
    covg[:, 1, 1] = covall[:, 1]
    covg[:, 2, 2] = covall[:, 2]
    covg[:, 0, 1] = covg[:, 1, 0] = covall[:, 3]
    covg[:, 0, 2] = covg[:, 2, 0] = covall[:, 4]
    covg[:, 1, 2] = covg[:, 2, 1] = covall[:, 5]
    _, vecs = np.linalg.eigh(covg)
    zl_all = np.ascontiguousarray(vecs[:, :, 0])
    _mark("eigh")

    zoff = 0
    for b in range(B):
        rows, cov6b, zsb, auxb, hidx_b, hS_b, hmap = samples[b]
        if rows.size == 0:
            continue
        zl = zl_all[zoff : zoff + rows.size]
        zoff += rows.size
        mg = 2 * auxb[:, 0]
        zeta = auxb[:, 1]
        flag = auxb[:, 2]
        cntd = auxb[:, 3]  # cnt - 128 (boundary-tie rows have cntd != 0)

        z0 = np.where(mg >= 0, 1.0, -1.0).astype(np.float32)[:, None] * zsb
        # remap device counts to the LAPACK orientation: pos(-z) = neg(z) + zeta
        sigma = np.einsum("rc,rc->r", zl, z0)
        pos = np.where(sigma >= 0, (mg + K) // 2, (K - mg) // 2 + zeta)
        # rows needing a true recount (unstable counts / unreliable device vec)
        rc = np.nonzero((flag >= 2) & (hmap[rows] >= 0))[0]
        if rc.size:
            slots = hmap[rows[rc]]
            nb = vertices[b][hidx_b[slots]] - vertices[b][rows[rc], None, :]
            zp = np.einsum("rkc,rc->rk", nb, zl[rc])
            pos[rc] = (zp >= 0).sum(axis=1)
        final = np.where((2 * pos - K >= 0)[:, None], zl, -zl)
        delta = (final - zsb).astype(np.float32)
        _mark(f"vote{b}")

        # boundary-tie rows (cnt != 128): the device mask summed extra points;
        # swap in the device-exported exact top-K-set sum.
        cntrows_l = np.nonzero(cntd != 0)[0]
        cntrows = rows[cntrows_l]
        for r in cntrows:
            sl = hmap[r]
            if sl >= 0:
                Sg[b][r] = hS_b[sl]

        # propagate corrections: row r is affected iff d2(r, m) <= r2k_r
        # BY DEVICE ARITHMETIC. Fast sgemm for the bulk test; pairs within a
        # narrow band of the threshold are re-decided with a bitwise mirror
        # of the device's elementwise d2 (same products, same association).
        big = np.abs(delta).max(axis=1) > 1e-3
        cols = rows[big]
        if cols.size:
            global _corr_buf, _corr_f32
            if _corr_buf is None:
                _corr_buf = np.empty((FMAX, N), np.float32)
                _corr_f32 = np.empty((FMAX, N), np.float32)
            F = cols.size
            dl = delta[big]
            if F > FMAX:  # degrade gracefully: drop the smallest deltas
                keep = np.argsort(-np.abs(dl).max(axis=1))[:FMAX]
                cols = cols[keep]
                dl = dl[keep]
                F = FMAX
            vb = vertices[b]
            vm = (vb[cols] * np.float32(-2.0)).astype(np.float32)  # fold -2 into the gemm
            tr = n2[b] - r2g[b]  # prefold per-row terms: d2 - thr = -2dot + n2m + (n2r - thr)
            # fast approximate containment via sgemm in [F, N] layout
            # (contiguous rows), exact bitwise device mirror only for pairs
            # within a narrow band of the threshold
            D2 = _corr_buf[:F]
            np.matmul(vm, vb.T, out=D2)
            D2 += n2[b][cols][:, None]
            D2 += tr[None, :]  # now D2 holds margin = d2_approx - thr
            basef = _corr_f32[:F]
            np.less_equal(D2, 0.0, out=basef, casting="unsafe")
            np.abs(D2, out=D2)
            band = D2 <= 2e-4  # sgemm-vs-mirror drift is <~1e-5; 20x margin
            hot = np.nonzero(band.sum(axis=1))[0]  # flagged cols with band pairs
            if hot.size:
                ari_l, aci_l = [], []
                for c in hot:
                    rr = np.nonzero(band[c])[0]
                    ari_l.append(rr)
                    aci_l.append(np.full(rr.size, c, np.int64))
                ari = np.concatenate(ari_l)
                aci = np.concatenate(aci_l)
                va, vc = vb[ari], vb[cols[aci]]
                # exact mirror: (p0 + p1) + p2, then (sq_q - 2 dot) + sq_all
                dot = (va[:, 0] * vc[:, 0] + va[:, 1] * vc[:, 1]) + va[:, 2] * vc[:, 2]
                d2x = (n2[b][ari] - np.float32(2.0) * dot) + n2[b][cols[aci]]
                basef[aci, ari] = (d2x <= r2g[b][ari]).astype(np.float32)
            # boundary-tie rows got the exact exported sum: apply their
            # corrections from the exported index row instead
            if cntrows.size:
                basef[:, cntrows] = 0.0
                colpos = np.full(N, -1, np.int32)
                colpos[cols] = np.arange(cols.size)
                for r in cntrows:
                    sl = hmap[r]
                    if sl >= 0:
                        cps = colpos[hidx_b[sl]]
                        for cp in cps[cps >= 0]:
                            Sg[b][r] += dl[cp]
            Sg[b] += basef.T @ dl
        # sub-threshold deltas (aligned, ~1e-6) are dropped: their effect on a
        # 128-normal average is < 1e-8
        _mark(f"corr{b}")

    nrm = np.sqrt(np.einsum("bnc,bnc->bn", Sg, Sg))
    Sg /= nrm[:, :, None]
    if _DEBUG_T:
        t3 = time.perf_counter()
        print(
            f"[kernel] dispatch {(t1-t0)*1e3:.1f}ms  sync+pull {(t2-t1)*1e3:.1f}ms"
            f"  host-fix {(t3-t2)*1e3:.1f}ms  "
            + " ".join(f"{k}={v*1e3:.1f}" for k, v in _tmarks),
            flush=True,
        )
    return Sg


# revision 6
# speedup vs baseline: 1.1966x; 1.0267x over previous
"""AveragedNormals on 8 Trainium2 NeuronCores — gather-free single-sync design.

Tunnel model (measured): every host<->device sync costs a fixed ~40-85ms RTT
(network-dependent), pull bandwidth ~77MB/s, chained dispatches are free.
Device-side indirect gathers are the other big cost (~55ms for the baseline's
two [2048x128] gathers), while lax.top_k on [2048, 8192] is only ~6ms.

So this kernel removes ALL device gathers by exploiting the SHOT weight
structure: w_j = radius - d_j is >= 0 exactly for the 128 nearest neighbors
and the weight of the 128th is exactly 0, so

  cov_n  = sum_j relu(radius_n - d_nj) x_nj x_nj^T   (x = v_j - q_n)
  vote_n = sum_j [d2_nj <= r2k_n] f(x_nj . z_n)
  S_n    = sum_j [d2_nj <= r2k_n] zsigned_j          (masked matmul)

over ALL 8192 points — identical term sets to the reference's gathered top-128
versions (only fp summation order differs, ~1e-7). Only the 128th-smallest
distance r2k per row is needed (top_k values; the index matrix is used solely
for the small hard-row export).

Correctness model (vs reference = top_k + LAPACK eigh + vote + gather-mean),
same as the baseline: ambiguous rows (vote ties in [0, 2*zeta], unstable
counts, weak eigengap, mask-count != 128) are exported compactly (cov6 + zs +
aux for <=448 rows/core) and fixed on host with np.linalg.eigh; corrections
delta_m = z_final - z_device propagate to every row whose neighborhood holds m
via a HOST-side distance matmul (vertices @ flipped^T vs pulled r2k) instead of
pulling per-row neighbor lists — cutting the pull from ~2.4MB to ~0.5MB.
"""

import functools
import os
import time

import jax
import jax.numpy as jnp
import numpy as np
from jax import lax

_DEBUG_T = bool(os.environ.get("AN_DEBUG_T"))
_DEBUG_FULL = bool(os.environ.get("AN_DEBUG_FULL"))
_tmarks = []

B = 2
N = 8192
K = 128
SPLIT = 4  # row-split per sample
NC = 8
ROWS = N // SPLIT  # 2048
EPS = 1e-12
FCAP = 512  # per-core flagged-row export capacity (tie rate is input-
# dependent: 5-10% observed across PRNG backends, theory ~14%; 512 covers 25%)
HARD = 48  # per-core exported hard-row (recount) capacity (measured max ~29)
HI = lax.Precision.HIGHEST
GROUPS = [[0, 1, 2, 3], [4, 5, 6, 7]]


def _smallest_evec_gap(cov):
    # cov: [R, 3, 3] symmetric. Unit eigenvector of the smallest eigenvalue
    # plus the relative gap (lam_mid - lam_min) / (lam_max - lam_min).
    a00 = cov[:, 0, 0]
    a01 = cov[:, 0, 1]
    a02 = cov[:, 0, 2]
    a11 = cov[:, 1, 1]
    a12 = cov[:, 1, 2]
    a22 = cov[:, 2, 2]

    q = (a00 + a11 + a22) / 3.0
    b00 = a00 - q
    b11 = a11 - q
    b22 = a22 - q
    p1 = a01 * a01 + a02 * a02 + a12 * a12
    p2 = b00 * b00 + b11 * b11 + b22 * b22 + 2.0 * p1
    p = jnp.sqrt(jnp.maximum(p2 / 6.0, 1e-30))
    detb = (
        b00 * (b11 * b22 - a12 * a12)
        - a01 * (a01 * b22 - a12 * a02)
        + a02 * (a01 * a12 - b11 * a02)
    )
    r = jnp.clip(detb / (2.0 * p * p * p), -1.0, 1.0)
    # acos via atan2 (mhlo.acos doesn't lower on the neuron backend)
    phi = jnp.arctan2(jnp.sqrt(jnp.maximum(1.0 - r * r, 0.0)), r) / 3.0
    lam_hi = q + 2.0 * p * jnp.cos(phi)
    lam = q + 2.0 * p * jnp.cos(phi + 2.0 * np.pi / 3.0)  # smallest
    lam_mid = 3.0 * q - lam_hi - lam
    spread = jnp.maximum(lam_hi - lam, 1e-30)
    gapr = (lam_mid - lam) / spread

    m00 = a00 - lam
    m11 = a11 - lam
    m22 = a22 - lam
    r0 = jnp.stack([m00, a01, a02], axis=-1)
    r1 = jnp.stack([a01, m11, a12], axis=-1)
    r2 = jnp.stack([a02, a12, m22], axis=-1)
    c01 = jnp.cross(r0, r1)
    c02 = jnp.cross(r0, r2)
    c12 = jnp.cross(r1, r2)
    n01 = jnp.sum(c01 * c01, axis=-1)
    n02 = jnp.sum(c02 * c02, axis=-1)
    n12 = jnp.sum(c12 * c12, axis=-1)
    best12 = (n12 >= n01) & (n12 >= n02)
    best02 = (n02 >= n01) & ~best12
    v = jnp.where(best12[:, None], c12, jnp.where(best02[:, None], c02, c01))
    nv = jnp.sqrt(jnp.maximum(jnp.sum(v * v, axis=-1, keepdims=True), 1e-30))
    v = v / nv

    # Two inverse-iteration refinements (Rayleigh quotient + adjugate solve):
    # the closed-form z is only ~1e-3 accurate; the vote is decided by
    # near-zero neighbor projections, so z must match eigh to ~1e-6.
    eps_reg = 1e-7 * jnp.maximum(jnp.abs(q), p)
    for _ in range(2):
        lam_r = (
            v[:, 0] * (a00 * v[:, 0] + a01 * v[:, 1] + a02 * v[:, 2])
            + v[:, 1] * (a01 * v[:, 0] + a11 * v[:, 1] + a12 * v[:, 2])
            + v[:, 2] * (a02 * v[:, 0] + a12 * v[:, 1] + a22 * v[:, 2])
        )
        m00 = a00 - lam_r + eps_reg
        m11 = a11 - lam_r + eps_reg
        m22 = a22 - lam_r + eps_reg
        y0 = (
            (m11 * m22 - a12 * a12) * v[:, 0]
            + (a02 * a12 - a01 * m22) * v[:, 1]
            + (a01 * a12 - a02 * m11) * v[:, 2]
        )
        y1 = (
            (a02 * a12 - a01 * m22) * v[:, 0]
            + (m00 * m22 - a02 * a02) * v[:, 1]
            + (a01 * a02 - m00 * a12) * v[:, 2]
        )
        y2 = (
            (a01 * a12 - a02 * m11) * v[:, 0]
            + (a01 * a02 - m00 * a12) * v[:, 1]
            + (m00 * m11 - a01 * a01) * v[:, 2]
        )
        y = jnp.stack([y0, y1, y2], axis=-1)
        y = jnp.where(jnp.sum(y * v, axis=-1, keepdims=True) < 0, -y, y)
        ny = jnp.sqrt(jnp.maximum(jnp.sum(y * y, axis=-1, keepdims=True), 1e-38))
        v = y / ny
    return v, gapr


def _stage1_body(v_sh):
    # v_sh: [ROWS, 3] this core's query block; replicate the full cloud via a
    # psum over the sample's 4-way split axis of disjoint zero-padded blocks
    # (bit-identical, 4x less push)
    vq = v_sh
    row0 = lax.axis_index("x") * ROWS
    vfp = jnp.zeros((N, 3), jnp.float32)
    vfp = lax.dynamic_update_slice(vfp, v_sh, (row0, 0))
    v_full = lax.psum(vfp, "x")

    # d2 via ELEMENTWISE ops in a fixed association order, bitwise-reproducible
    # by numpy on the host: the host correction step must replicate the
    # device's neighborhood-membership test d2 <= r2k exactly, and a matmul's
    # accumulation order can't be reproduced host-side. Mirror this exact
    # op sequence (products, left-assoc sums) in kernel()'s containment test.
    a0 = v_full[:, 0]
    a1 = v_full[:, 1]
    a2 = v_full[:, 2]
    sq_all = (a0 * a0 + a1 * a1) + a2 * a2  # [N]
    q0 = vq[:, 0]
    q1 = vq[:, 1]
    q2 = vq[:, 2]
    sq_q = (q0 * q0 + q1 * q1) + q2 * q2  # [ROWS]
    dot = (q0[:, None] * a0[None, :] + q1[:, None] * a1[None, :]) + q2[:, None] * a2[None, :]
    d2 = (sq_q[:, None] - 2.0 * dot) + sq_all[None, :]  # [ROWS, N]
    # One materialization of d2 shared by top_k / mask / weights: XLA would
    # otherwise rematerialize it per consumer fusion with different rounding,
    # silently swapping boundary neighbors between the top-k set and the mask.
    d2 = lax.optimization_barrier(d2)

    # sqrt is monotone, so top-k of -d2 selects the same neighbor set as the
    # reference's top-k of -sqrt(max(d2, EPS)) (the EPS plateau only merges
    # the self-distance, which is in the set either way).
    neg2, idx = lax.top_k(-d2, K)
    r2k = -neg2[:, -1]  # [ROWS] squared distance of the 128th-nearest
    radius = jnp.sqrt(jnp.maximum(r2k, EPS))

    # gather-free masked reductions over all N points
    x0 = v_full[:, 0][None, :] - vq[:, 0][:, None]  # [ROWS, N]
    x1 = v_full[:, 1][None, :] - vq[:, 1][:, None]
    x2 = v_full[:, 2][None, :] - vq[:, 2][:, None]
    d = jnp.sqrt(jnp.maximum(d2, EPS))
    w = jnp.maximum(radius[:, None] - d, 0.0)  # SHOT weight; 0 outside top-K
    mask = (d2 <= r2k[:, None]).astype(jnp.float32)  # [ROWS, N]

    cnt = jnp.sum(mask, axis=-1)  # 128 unless d2 ties at the boundary
    wsum = jnp.sum(w, axis=-1)
    c00 = jnp.sum(w * x0 * x0, axis=-1)
    c11 = jnp.sum(w * x1 * x1, axis=-1)
    c22 = jnp.sum(w * x2 * x2, axis=-1)
    c01 = jnp.sum(w * x0 * x1, axis=-1)
    c02 = jnp.sum(w * x0 * x2, axis=-1)
    c12 = jnp.sum(w * x1 * x2, axis=-1)
    iw = 1.0 / wsum
    cov6 = jnp.stack([c00, c11, c22, c01, c02, c12], axis=-1) * iw[:, None]
    cov = jnp.stack(
        [
            jnp.stack([c00, c01, c02], axis=-1),
            jnp.stack([c01, c11, c12], axis=-1),
            jnp.stack([c02, c12, c22], axis=-1),
        ],
        axis=1,
    ) * iw[:, None, None]

    z0, gapr = _smallest_evec_gap(cov)  # [ROWS, 3], [ROWS]

    # SHOT sign vote with the device eigenvector, masked over all N
    zp = x0 * z0[:, 0][:, None] + x1 * z0[:, 1][:, None] + x2 * z0[:, 2][:, None]
    posc = jnp.sum(mask * (zp >= 0), axis=-1).astype(jnp.int32)
    zeta = jnp.sum(mask * (zp == 0), axis=-1).astype(jnp.int32)
    abszp = jnp.where((mask > 0) & (zp != 0), jnp.abs(zp), jnp.float32(np.inf))
    minabs = jnp.min(abszp, axis=-1)
    margin = 2 * posc - K
    s = jnp.where(margin >= 0, 1.0, -1.0).astype(jnp.float32)
    zs = s[:, None] * z0  # vote-oriented device normal

    # ambiguity flags (host fixes these rows with LAPACK eigh)
    f_tie = (margin >= 0) & (margin <= 2 * zeta)  # LAPACK sign decides
    f_zp = (
        (minabs < 3e-5 * radius) & (margin >= -4) & (margin <= 2 * zeta + 4)
    ) | (zeta > 1)  # counts unstable near a boundary (z0 error ~1e-6)
    f_gap = gapr < 3e-3  # device eigenvector unreliable
    f_cnt = cnt != 128.0  # boundary tie: mask picked up extras
    recount = f_zp | f_gap | f_cnt
    flag = f_tie.astype(jnp.int32) + 2 * recount.astype(jnp.int32)

    # replicate signed normals across the sample's 4-core group
    zfull = jnp.zeros((N, 3), jnp.float32)
    zfull = lax.dynamic_update_slice(zfull, zs, (row0, 0))
    zfull = lax.psum(zfull, "x")

    # neighbor-average numerator via masked matmul (no gather)
    S = lax.dot_general(mask, zfull, (((1,), (0,)), ((), ())), precision=HI)

    # compacted flagged-row export (f32 scores: TopK rejects integer inputs)
    fsc = (flag > 0).astype(jnp.float32) * 100000.0 + jnp.arange(
        ROWS, dtype=jnp.float32
    )
    fval, frow = lax.top_k(fsc, FCAP)
    frows = jnp.where(fval >= 100000.0, frow, -1).astype(jnp.int16)
    fcov6 = jnp.take(cov6, frow, axis=0)  # [FCAP, 6]
    fzs = jnp.take(zs, frow, axis=0)  # [FCAP, 3]
    faux = jnp.stack(
        [
            jnp.take(margin, frow) // 2,  # margin even: store margin/2
            jnp.minimum(jnp.take(zeta, frow), 127),
            jnp.take(flag, frow),
            jnp.clip(jnp.take(cnt.astype(jnp.int32), frow) - K, -8, 8),
        ],
        axis=-1,
    ).astype(jnp.int8)

    # export top-K index rows for rows needing a host vote recount
    hsc = recount.astype(jnp.float32) * 100000.0 + jnp.arange(
        ROWS, dtype=jnp.float32
    )
    hval, hrow = lax.top_k(hsc, HARD)
    hard_rows = jnp.where(hval >= 100000.0, hrow, -1).astype(jnp.int16)
    hard_idx = jnp.take(idx, hrow, axis=0).astype(jnp.int16)  # [HARD, K]
    # exact top-K-set sum for hard rows (tiny gather): boundary-tie rows'
    # masked S picked up extras; the host swaps in this exact sum instead
    hard_S = jnp.take(zfull, jnp.take(idx, hrow, axis=0).reshape(-1), axis=0)
    hard_S = jnp.sum(hard_S.reshape(HARD, K, 3), axis=1)  # [HARD, 3]

    # pack into few arrays: per-array-per-shard pull overhead is measurable
    main = jnp.concatenate([S, r2k[:, None]], axis=1)  # [ROWS, 4] f32
    fblk = jnp.concatenate(
        [
            fcov6,
            fzs,
            frows.astype(jnp.float32)[:, None],  # int16 values, exact in f32
            faux.astype(jnp.float32),
        ],
        axis=1,
    )  # [FCAP, 14] f32
    hblk = jnp.concatenate([hard_rows[:, None], hard_idx], axis=1)  # [HARD, 129] i16
    if _DEBUG_FULL:
        return main, fblk, hblk, hard_S, zs, jnp.stack([margin, zeta, flag], -1), cov6, gapr[:, None], minabs[:, None], idx.astype(jnp.int16), cnt[:, None]
    return main, fblk, hblk, hard_S


_corr_buf = None  # preallocated [FMAX, N] f32 scratch for the host correction
_corr_f32 = None
FMAX = 1024  # max big-delta columns per sample (observed up to ~430)
_mesh = None
_stage1 = None


def _build():
    global _mesh, _stage1
    from jax.experimental.shard_map import shard_map
    from jax.sharding import Mesh, PartitionSpec as P

    devs = np.array(jax.devices()[:NC]).reshape(B, SPLIT)
    _mesh = Mesh(devs, ("b", "x"))

    def body(v_blk):
        # v_blk: [1, 1, ROWS, 3] block of [B, SPLIT, ROWS, 3]
        outs = _stage1_body(v_blk[0, 0])
        return tuple(o[None, None] for o in outs)

    nout = 11 if _DEBUG_FULL else 4
    sm = shard_map(
        body,
        mesh=_mesh,
        in_specs=(P("b", "x", None, None),),
        out_specs=tuple(P("b", "x", None, None) for _ in range(nout)),
    )
    _stage1 = jax.jit(sm)


def kernel(vertices: np.ndarray) -> np.ndarray:
    vertices = np.asarray(vertices, dtype=np.float32)
    assert vertices.shape == (B, N, 3)
    v_sh = vertices.reshape(B, SPLIT, ROWS, 3)

    t0 = time.perf_counter()
    if _stage1 is None:
        _build()
    outs = _stage1(jnp.asarray(v_sh))
    t1 = time.perf_counter()
    pulled = jax.device_get(outs)
    t2 = time.perf_counter()
    global _last_pull
    _last_pull = pulled
    main = pulled[0].reshape(NC, ROWS, 4)
    fblk = pulled[1].reshape(NC, FCAP, 14)
    hblk = pulled[2].reshape(NC, HARD, 129)
    hard_S = pulled[3].reshape(NC, HARD, 3)
    S = main[:, :, :3]
    r2k = main[:, :, 3]
    fcov6 = fblk[:, :, 0:6]
    fzs = fblk[:, :, 6:9]
    frows_f = fblk[:, :, 9]
    faux_f = fblk[:, :, 10:14]
    hard_rows = hblk[:, :, 0]
    hard_idx = hblk[:, :, 1:]

    _tmarks.clear()
    tp = time.perf_counter()

    def _mark(name):
        nonlocal tp
        now = time.perf_counter()
        _tmarks.append((name, now - tp))
        tp = now

    # core c -> sample c//4, rows [(c%4)*ROWS, ...): plain reshape restores [B,N]
    Sg = np.array(S.reshape(B, N, 3))  # writable copy
    r2g = r2k.reshape(B, N)
    # n2 mirrors the device's sq_all association order exactly
    n2 = (vertices[..., 0] * vertices[..., 0] + vertices[..., 1] * vertices[..., 1]) + vertices[..., 2] * vertices[..., 2]

    # hard-row maps (vectorized): per-core flagged slots sort first
    nhard = (hard_rows >= 0).sum(axis=1)  # [NC]
    _mark("unpack")

    # phase 1: unpack per-sample flagged exports
    samples = []
    for b in range(B):
        rows_l, cov_l, zs_l, aux_l = [], [], [], []
        hmap = np.full(N, -1, np.int32)
        hidx_l, hS_l = [], []
        hn = 0
        for c in range(b * SPLIT, (b + 1) * SPLIT):
            nv = int((frows_f[c] >= 0).sum())
            off = (c % SPLIT) * ROWS
            rows_l.append(frows_f[c, :nv].astype(np.int32) + off)
            cov_l.append(fcov6[c, :nv])
            zs_l.append(fzs[c, :nv])
            aux_l.append(faux_f[c, :nv])
            nh = int(nhard[c])
            hmap[hard_rows[c, :nh].astype(np.int32) + off] = hn + np.arange(nh)
            hidx_l.append(hard_idx[c, :nh].astype(np.int32))
            hS_l.append(hard_S[c, :nh])
            hn += nh
        if _DEBUG_T:
            nv_pc = [int((frows_f[c] >= 0).sum()) for c in range(b * SPLIT, (b + 1) * SPLIT)]
            print(f"[kernel] sample {b}: flagged/core {nv_pc} (cap {FCAP}), hard {hn} (cap {HARD}/core)", flush=True)
        samples.append(
            (
                np.concatenate(rows_l),
                np.concatenate(cov_l).astype(np.float32),
                np.concatenate(zs_l).astype(np.float32),
                np.concatenate(aux_l).astype(np.int32),
                np.concatenate(hidx_l) if hn else np.zeros((0, K), np.int32),
                np.concatenate(hS_l) if hn else np.zeros((0, 3), np.float32),
                hmap,
            )
        )
    _mark("gather")

    # one merged LAPACK eigh across both samples: its sign convention is the spec
    covall = np.concatenate([s[1] for s in samples], axis=0)
    covg = np.empty((covall.shape[0], 3, 3), np.float32)
    covg[:, 0, 0] = covall[:, 0]
    covg[:, 1, 1] = covall[:, 1]
    covg[:, 2, 2] = covall[:, 2]
    covg[:, 0, 1] = covg[:, 1, 0] = covall[:, 3]
    covg[:, 0, 2] = covg[:, 2, 0] = covall[:, 4]
    covg[:, 1, 2] = covg[:, 2, 1] = covall[:, 5]
    _, vecs = np.linalg.eigh(covg)
    zl_all = np.ascontiguousarray(vecs[:, :, 0])
    _mark("eigh")

    zoff = 0
    for b in range(B):
        rows, cov6b, zsb, auxb, hidx_b, hS_b, hmap = samples[b]
        if rows.size == 0:
            continue
        zl = zl_all[zoff : zoff + rows.size]
        zoff += rows.size
        mg = 2 * auxb[:, 0]
        zeta = auxb[:, 1]
        flag = auxb[:, 2]
        cntd = auxb[:, 3]  # cnt - 128 (boundary-tie rows have cntd != 0)

        z0 = np.where(mg >= 0, 1.0, -1.0).astype(np.float32)[:, None] * zsb
        # remap device counts to the LAPACK orientation: pos(-z) = neg(z) + zeta
        sigma = np.einsum("rc,rc->r", zl, z0)
        pos = np.where(sigma >= 0, (mg + K) // 2, (K - mg) // 2 + zeta)
        # rows needing a true recount (unstable counts / unreliable device vec)
        rc = np.nonzero((flag >= 2) & (hmap[rows] >= 0))[0]
        if rc.size:
            slots = hmap[rows[rc]]
            nb = vertices[b][hidx_b[slots]] - vertices[b][rows[rc], None, :]
            zp = np.einsum("rkc,rc->rk", nb, zl[rc])
            pos[rc] = (zp >= 0).sum(axis=1)
        final = np.where((2 * pos - K >= 0)[:, None], zl, -zl)
        delta = (final - zsb).astype(np.float32)
        _mark(f"vote{b}")

        # boundary-tie rows (cnt != 128): the device mask summed extra points;
        # swap in the device-exported exact top-K-set sum.
        cntrows_l = np.nonzero(cntd != 0)[0]
        cntrows = rows[cntrows_l]
        for r in cntrows:
            sl = hmap[r]
            if sl >= 0:
                Sg[b][r] = hS_b[sl]

        # propagate corrections: row r is affected iff d2(r, m) <= r2k_r
        # BY DEVICE ARITHMETIC. Fast sgemm for the bulk test; pairs within a
        # narrow band of the threshold are re-decided with a bitwise mirror
        # of the device's elementwise d2 (same products, same association).
        big = np.abs(delta).max(axis=1) > 1e-3
        cols = rows[big]
        if cols.size:
            global _corr_buf, _corr_f32
            if _corr_buf is None:
                _corr_buf = np.empty((FMAX, N), np.float32)
                _corr_f32 = np.empty((FMAX, N), np.float32)
            F = cols.size
            dl = delta[big]
            if F > FMAX:  # degrade gracefully: drop the smallest deltas
                keep = np.argsort(-np.abs(dl).max(axis=1))[:FMAX]
                cols = cols[keep]
                dl = dl[keep]
                F = FMAX
            vb = vertices[b]
            # single K=5 augmented gemm computes the whole approximate margin
            # matrix d2 - thr = -2 dot + n2m + (n2r - thr) in one BLAS pass
            A = np.empty((F, 5), np.float32)
            A[:, :3] = vb[cols] * np.float32(-2.0)
            A[:, 3] = n2[b][cols]
            A[:, 4] = 1.0
            Bm = np.empty((N, 5), np.float32)
            Bm[:, :3] = vb
            Bm[:, 3] = 1.0
            Bm[:, 4] = n2[b] - r2g[b]
            D2 = _corr_buf[:F]
            np.matmul(A, Bm.T, out=D2)  # D2 holds margin = d2_approx - thr
            basef = _corr_f32[:F]
            np.less_equal(D2, 0.0, out=basef, casting="unsafe")
            np.abs(D2, out=D2)
            band = D2 <= 2e-4  # sgemm-vs-mirror drift is <~1e-5; 20x margin
            hot = np.nonzero(band.sum(axis=1))[0]  # flagged cols with band pairs
            if hot.size:
                ari_l, aci_l = [], []
                for c in hot:
                    rr = np.nonzero(band[c])[0]
                    ari_l.append(rr)
                    aci_l.append(np.full(rr.size, c, np.int64))
                ari = np.concatenate(ari_l)
                aci = np.concatenate(aci_l)
                va, vc = vb[ari], vb[cols[aci]]
                # exact mirror: (p0 + p1) + p2, then (sq_q - 2 dot) + sq_all
                dot = (va[:, 0] * vc[:, 0] + va[:, 1] * vc[:, 1]) + va[:, 2] * vc[:, 2]
                d2x = (n2[b][ari] - np.float32(2.0) * dot) + n2[b][cols[aci]]
                basef[aci, ari] = (d2x <= r2g[b][ari]).astype(np.float32)
            # boundary-tie rows got the exact exported sum: apply their
            # corrections from the exported index row instead
            if cntrows.size:
                basef[:, cntrows] = 0.0
                colpos = np.full(N, -1, np.int32)
                colpos[cols] = np.arange(cols.size)
                for r in cntrows:
                    sl = hmap[r]
                    if sl >= 0:
                        cps = colpos[hidx_b[sl]]
                        for cp in cps[cps >= 0]:
                            Sg[b][r] += dl[cp]
            Sg[b] += basef.T @ dl
        # sub-threshold deltas (aligned, ~1e-6) are dropped: their effect on a
        # 128-normal average is < 1e-8
        _mark(f"corr{b}")

    nrm = np.sqrt(np.einsum("bnc,bnc->bn", Sg, Sg))
    Sg /= nrm[:, :, None]
    if _DEBUG_T:
        t3 = time.perf_counter()
        print(
            f"[kernel] dispatch {(t1-t0)*1e3:.1f}ms  sync+pull {(t2-t1)*1e3:.1f}ms"
            f"  host-fix {(t3-t2)*1e3:.1f}ms  "
            + " ".join(f"{k}={v*1e3:.1f}" for k, v in _tmarks),
            flush=True,
        )
    return Sg
